# revision 40
# baseline (speedup 1.0000x reference)
"""Trainium2 Bass kernel for nn_AugmentedODE (B=64, N=P=512), 8-core data parallel.

Per batch the reference computes (7 matmuls of 512^3):
    Omega   = 0.5*(A - A^T)
    du      = u @ Omega + G - u @ (u^T G)
    S       = lam @ G^T
    dlam    = lam @ A + (S + S^T) @ u

Restructured to 5 matmuls + 1 PE transpose set per batch:
    UTG = u^T G                      (bf16:  lhsT=u,    rhs=G)
    W   = 0.5*(A - A^T) - UTG        (DVE; A - A^T precomputed host-side, fp8)
    du  = u @ W + G                  (bf16:  lhsT=u^T,  rhs=W; +G fused in DVE)
    S   = lam @ G^T                  (bf16:  lhsT=lam^T, rhs=G^T)
    C   = S + S^T                    (bf16 PE transpose + DVE add)
    dlam= lam @ A + C @ u            (lam@A in fp8 DoubleRow at 2x rate; C@u bf16;
                                      both accumulated into one PSUM group)

Rel-err budget is 2e-2 (Frobenius); measured ~1.7e-2 for this mix.  The four
magnitude-dominant matmuls (UTG, u@W, S, C@u) stay bf16 for batches 2-7 and
run at the PE's bf16 roofline (1 row/cycle, ~216 ns per 128x128x512 matmul).
Batch 0 is fully fp8 (its head DMA is halved); batch 1 additionally runs its
UTG and S matmuls in fp8 DoubleRow (2 k-blocks per pass, ~1.6us PE saved
each), which costs ~+0.4e-2 on each output — still under the gate.  lam@A
(~3% of |dlam|) and the skew term (~5% of |W|) are fp8 DR everywhere.

(Note: offloading the S+S^T transposes to the DMA xbar was tried and
reverted — Tile globally serializes dma-transposes against ALL in-flight
HWDGE DMAs as a HW-deadlock guard, which strings them out by 15-25us and
starves the PE.  The 16 PE transposes per batch stream at ~56ns each and
are the cheapest correct option.)

All operands are pre-packed on the host into the exact SBUF layout
([128 partitions, kblock, 512] with k-blocks contiguous per partition) and
concatenated into three blobs per batch, so every DMA line is multi-KB
contiguous on both sides.  Dependency tracking is per-tile, so batch 0 uses
peeled per-piece tiles whose DMAs are sequenced on one queue in consumption
order; batches 1-3 stream whole blobs on the same queue behind them; later
batches prefetch on parallel queues, gated naturally by the 4-deep input
rings.  Batches are processed in PAIRS with sections interleaved
(M1(b), M1(b+1), M5(b), M5(b+1), ...) so every section boundary is followed
by independent work and cross-engine handoff latencies stay off the PE
critical path.  Outputs are written bf16 and upcast on the host.
"""
import numpy as np
import ml_dtypes

import concourse.bass as bass
import concourse.mybir as mybir
import concourse.tile as tile
from concourse import bacc
from concourse.bass_utils import run_bass_kernel_spmd
from concourse.masks import make_identity

F32 = mybir.dt.float32
F32R = mybir.dt.float32r
BF16 = mybir.dt.bfloat16
F8 = mybir.dt.float8e4
AOP = mybir.AluOpType
DR = mybir.MatmulPerfMode.DoubleRow

NP_BF16 = ml_dtypes.bfloat16
NP_F8 = ml_dtypes.float8_e4m3

B, N, P = 64, 512, 512
NCORES = 8
BLOC = B // NCORES          # batches per core
KB = 4                      # 512 = 4 k-blocks of 128
CH = 4                      # 4 output chunks of 128 rows
WARMUP_MM = 6


def _build_nc():
    nc = bacc.Bacc("TRN2", target_bir_lowering=False, debug=False,
                   num_devices=NCORES)

    # in1: interleaved [u0,g0,u1,g1,u2,g2,u3,g3]
    # in2: interleaved [lamt0,gt0,...,lamt3,gt3, ut0..ut3]
    # in8: amat=A-A^T(0:4) | a(4:8) | lamt8(8:12), fp8e4
    d_in1 = nc.declare_dram_parameter("in1", [BLOC, 128, 2 * KB, P], BF16,
                                      isOutput=False)
    d_in2 = nc.declare_dram_parameter("in2", [BLOC, 128, 3 * KB, P], BF16,
                                      isOutput=False)
    d_in8 = nc.declare_dram_parameter("in8", [BLOC, 128, 3 * KB, P], F8,
                                      isOutput=False)
    # batch-0 fp8 head blobs: h2f8 = [gt0..3 | ut0..3] (its lamt fp8 comes
    # from in8's lamt8 blocks, so it is not shipped twice), h1f8 =
    # [u0,g0 | u1,g1 | u2,u3,g2,g3] (last four blocks reordered so the
    # (u2,u3)x(g2,g3) k-pair runs as one fp8 DoubleRow pass)
    d_h2f8 = nc.declare_dram_parameter("h2f8", [128, 2 * KB, P], F8,
                                       isOutput=False)
    d_h1f8 = nc.declare_dram_parameter("h1f8", [128, 2 * KB, P], F8,
                                       isOutput=False)
    # batch-1 fp8 operands for its DoubleRow UTG, S and u@W matmuls:
    # [u8(0:4) | g8(4:8) | gt8(8:12) | ut8(12:16)]
    d_x8b1 = nc.declare_dram_parameter("x8b1", [128, 4 * KB, P], F8,
                                       isOutput=False)
    d_du = nc.declare_dram_parameter("du", [BLOC, 128, KB, P], BF16,
                                     isOutput=True)
    d_dlam = nc.declare_dram_parameter("dlam", [BLOC, 128, KB, P], BF16,
                                       isOutput=True)

    with tile.TileContext(nc) as tc:
        with (
            tc.tile_pool(name="const", bufs=1) as constp,
            tc.tile_pool(name="ins", bufs=4) as insp,
            tc.tile_pool(name="mid", bufs=2) as midp,
            tc.tile_pool(name="outs", bufs=2) as outsp,
            tc.tile_pool(name="psum", bufs=6, space="PSUM") as psum,
        ):
            # warm-up source first: one memset, no identity dependency, so the
            # PE's ~6us low-clock window starts at the earliest possible point
            # memset on DVE: gpsimd is busy with its framework preamble
            # (DMA rings, library loads) until ~7us, which would delay the
            # first warm-up matmul and with it the HAM un-throttle point
            wsrc = constp.tile([128, 512], BF16)
            nc.vector.memset(wsrc[:].bitcast(F32), 0.0)
            warm_ps = psum.tile([128, 512], F32, tag="ps")
            for i in range(WARMUP_MM):
                nc.tensor.matmul(warm_ps[:], wsrc[:, 0:128], wsrc[:],
                                 start=True, stop=True)

            identf = constp.tile([128, 128], F32)
            make_identity(nc, identf[:])
            identb = constp.tile([128, 128], BF16)
            nc.gpsimd.tensor_copy(identb[:], identf[:])

            # batch-0 peeled tiles: per-piece DMAs on ONE queue in consumption
            # order (dep tracking is per-tile, so fine pieces start compute early)
            h1a = insp.tile([128, 2, P], F8, tag="h1a", bufs=1)        # u0,g0
            h1b1 = insp.tile([128, 2, P], F8, tag="h1b1", bufs=1)      # u1,g1
            h1b2 = insp.tile([128, 4, P], F8, tag="h1b2", bufs=1)      # u2,u3,g2,g3
            h8a = insp.tile([128, KB, P], F8, tag="h8a", bufs=1)       # amat
            hgt = insp.tile([128, KB, P], F8, tag="hgt", bufs=1)       # gt0..3
            h8b2 = insp.tile([128, KB, P], F8, tag="h8b2", bufs=1)     # lamt8
            h2c = insp.tile([128, KB, P], F8, tag="h2c", bufs=1)       # ut
            h8b1 = insp.tile([128, KB, P], F8, tag="h8b1", bufs=1)     # a
            nc.sync.dma_start(h1a[:], d_h1f8[:, 0:2])
            nc.sync.dma_start(h1b1[:], d_h1f8[:, 2:4])
            nc.sync.dma_start(h1b2[:], d_h1f8[:, 4:2 * KB])
            nc.sync.dma_start(h8a[:], d_in8[0][:, 0:KB])
            nc.sync.dma_start(hgt[:], d_h2f8[:, 0:KB])
            nc.sync.dma_start(h8b2[:], d_in8[0][:, 2 * KB:3 * KB])
            nc.sync.dma_start(h2c[:], d_h2f8[:, KB:2 * KB])
            nc.sync.dma_start(h8b1[:], d_in8[0][:, KB:2 * KB])

            def mk_views(b):
                """Allocate input tiles (b>0), emit their DMAs, return accessors."""
                if b == 0:
                    h1 = [h1a, h1b1, h1b2, h1b2]
                    h1o = [0, 0, 0, 1]
                    g1o = [1, 1, 2, 3]
                    return dict(
                        Un=lambda k: h1[k][:, h1o[k]],
                        Gn=lambda k: h1[k][:, g1o[k]],
                        UT=lambda k: h2c[:, k],
                        AM8=lambda k: h8a[:, k],
                        A8pair=lambda j: h8b1[:, 2 * j:2 * j + 2],
                        L8pair=lambda j, r: h8b2[:, 2 * j:2 * j + 2,
                                                 r * 128:(r + 1) * 128],
                        GT8pair=lambda j: hgt[:, 2 * j:2 * j + 2],
                        U8H=lambda r: h1b2[:, 0:2, r * 128:(r + 1) * 128],
                        G8H=lambda: h1b2[:, 2:4],
                    )
                if b == 1:
                    # batch 1 computes UTG, S, AND u@W in fp8 DoubleRow from
                    # x8b1 (= [u8 | g8 | gt8 | ut8]); its bf16 lamt/gt/ut are
                    # never read, nor is its bf16 G (the du "+G" add reads
                    # g8 — G is ~1/512 of |du|, so fp8 G costs nothing).
                    # Only the u half of in1[1] is still loaded in bf16 (the
                    # C@u right operand, which feeds the dominant dlam term).
                    # The DR sections consume bytes faster than HBM delivers,
                    # so the blobs are peeled into per-section pieces queued
                    # in exact consumption order: M1's u8+g8, M5's gt8+lamt8,
                    # W's amat, M23's ut8, then the tail operands.
                    x8m = insp.tile([128, 2 * KB, P], F8, tag="x8m", bufs=1)
                    x8gt = insp.tile([128, KB, P], F8, tag="x8gt", bufs=1)
                    x8ut = insp.tile([128, KB, P], F8, tag="x8ut", bufs=1)
                    in8L = insp.tile([128, KB, P], F8, tag="in8L", bufs=1)
                    in8am = insp.tile([128, KB, P], F8, tag="in8am", bufs=1)
                    in8a = insp.tile([128, KB, P], F8, tag="in8a", bufs=1)
                    in1u = insp.tile([128, KB, P], BF16, tag="in1b1", bufs=1)
                    nc.sync.dma_start(x8m[:], d_x8b1[:, 0:2 * KB])
                    nc.sync.dma_start(x8gt[:], d_x8b1[:, 2 * KB:3 * KB])
                    nc.sync.dma_start(in8L[:], d_in8[1][:, 2 * KB:3 * KB])
                    nc.sync.dma_start(in8am[:], d_in8[1][:, 0:KB])
                    nc.sync.dma_start(x8ut[:], d_x8b1[:, 3 * KB:4 * KB])
                    nc.sync.dma_start(in1u[:], d_in1[1][:, 0:2 * KB:2])
                    nc.sync.dma_start(in8a[:], d_in8[1][:, KB:2 * KB])
                    return dict(
                        Un=lambda k, t=in1u: t[:, k],
                        Gn=lambda k, t=x8m: t[:, KB + k],
                        AM8=lambda k, t=in8am: t[:, k],
                        A8pair=lambda j, t=in8a: t[:, 2 * j:2 * j + 2],
                        L8pair=lambda j, r, t=in8L: t[:, 2 * j:2 * j + 2,
                                                      r * 128:(r + 1) * 128],
                        U8pair=lambda j, r, t=x8m: t[:, 2 * j:2 * j + 2,
                                                     r * 128:(r + 1) * 128],
                        G8pair=lambda j, t=x8m: t[:, KB + 2 * j:KB + 2 * j + 2],
                        GT8pair=lambda j, t=x8gt: t[:, 2 * j:2 * j + 2],
                        UT8pair=lambda j, r, t=x8ut: t[:, 2 * j:2 * j + 2,
                                                      r * 128:(r + 1) * 128],
                    )
                # bufs=3: only b2..b7 allocate from these rings (b1 uses
                # private slim tiles), so 3 slots restore the "b5 gated on
                # b2's release" back-pressure that keeps late batches' loads
                # from stealing head DMA bandwidth.
                in1 = insp.tile([128, 2 * KB, P], BF16, tag="in1", bufs=3,
                                name=f"in1_{b}")
                in2 = insp.tile([128, 3 * KB, P], BF16, tag="in2", bufs=3,
                                name=f"in2_{b}")
                in8 = insp.tile([128, 3 * KB, P], F8, tag="in8",
                                name=f"in8_{b}")
                if b <= 4:
                    # head batches: ordered on sync behind batch 0's pieces,
                    # so nothing dilutes the stream batch 0/1 are waiting on
                    nc.sync.dma_start(in1[:], d_in1[b])
                    nc.sync.dma_start(in8[:], d_in8[b])
                    nc.sync.dma_start(in2[:], d_in2[b])
                else:
                    # b>=5 are ring-gated behind live tiles (slot frees only
                    # after batch b-4 completes), so parallel queues can't
                    # steal head bandwidth
                    nc.sync.dma_start(in1[:], d_in1[b])
                    nc.gpsimd.dma_start(in2[:], d_in2[b])
                    nc.gpsimd.dma_start(in8[:], d_in8[b])
                return dict(
                    Un=lambda k, t=in1: t[:, 2 * k],
                    Gn=lambda k, t=in1: t[:, 2 * k + 1],
                    LT=lambda k, t=in2: t[:, 2 * k],
                    GT=lambda k, t=in2: t[:, 2 * k + 1],
                    UT=lambda k, t=in2: t[:, 2 * KB + k],
                    AM8=lambda k, t=in8: t[:, k],
                    A8pair=lambda j, t=in8: t[:, KB + 2 * j:KB + 2 * j + 2],
                    L8pair=lambda j, r, t=in8: t[:, 2 * KB + 2 * j:
                                                 2 * KB + 2 * j + 2,
                                                 r * 128:(r + 1) * 128],
                )

            def sec_m1(b, V):
                """M1: UTG = u^T G (k-outer) ; W = 0.5*amat - UTG (DVE)."""
                if "UT8pair" in V:
                    # batch 1: W is written in fp8 so u@W can run DoubleRow
                    # (W's k-blocks are pair-adjacent in this layout)
                    w_sb = midp.tile([128, KB, P], F8, tag="w8", bufs=1,
                                     name=f"w8_{b}")
                else:
                    w_sb = midp.tile([128, KB, P], BF16, tag="w",
                                     name=f"w_{b}")
                utg = [psum.tile([128, P], F32, tag="ps", name=f"utg{b}_{r}")
                       for r in range(CH)]
                if "U8pair" in V:
                    for j in range(2):
                        for r in range(CH):
                            nc.tensor.matmul(utg[r][:], V["U8pair"](j, r),
                                             V["G8pair"](j), perf_mode=DR,
                                             start=(j == 0), stop=(j == 1))
                elif "U8H" in V:
                    # batch 0 hybrid: k=0,1 as plain fp8 passes (each gated on
                    # a 128KB head piece, so compute starts at first-piece
                    # arrival), then the (u2,u3) k-pair as one DR pass
                    for k in range(2):
                        for r in range(CH):
                            nc.tensor.matmul(utg[r][:],
                                             V["Un"](k)[:, r * 128:(r + 1) * 128],
                                             V["Gn"](k)[:], start=(k == 0),
                                             stop=False)
                    for r in range(CH):
                        nc.tensor.matmul(utg[r][:], V["U8H"](r), V["G8H"](),
                                         perf_mode=DR, start=False, stop=True,
                                         skip_group_check=True)
                else:
                    for k in range(KB):
                        for r in range(CH):
                            nc.tensor.matmul(utg[r][:],
                                             V["Un"](k)[:, r * 128:(r + 1) * 128],
                                             V["Gn"](k)[:], start=(k == 0),
                                             stop=(k == KB - 1))
                # batch 1: u8 is shipped pre-scaled by 1/4 (exact exponent
                # shift), so utg = UTG/4 and W8 = W/4 stays inside fp8e4's
                # +-240 range (|W| itself reaches ~590); the du add scales
                # the matmul result back by 4.
                wscale = 0.125 if "UT8pair" in V else 0.5
                for r in range(CH):
                    nc.vector.scalar_tensor_tensor(w_sb[:, r], V["AM8"](r)[:],
                                                   wscale, utg[r][:], AOP.mult,
                                                   AOP.subtract)
                return w_sb

            def sec_m5(b, V):
                """M5: S = lam @ G^T (k-outer)."""
                s_sb = midp.tile([128, KB, N], BF16, tag="s", name=f"s_{b}")
                s_ps = [psum.tile([128, N], F32, tag="ps", name=f"s{b}_{r}")
                        for r in range(CH)]
                if "GT8pair" in V:
                    for j in range(2):
                        for r in range(CH):
                            nc.tensor.matmul(s_ps[r][:], V["L8pair"](j, r),
                                             V["GT8pair"](j), perf_mode=DR,
                                             start=(j == 0), stop=(j == 1))
                else:
                    for k in range(KB):
                        for r in range(CH):
                            nc.tensor.matmul(s_ps[r][:],
                                             V["LT"](k)[:, r * 128:(r + 1) * 128],
                                             V["GT"](k)[:], start=(k == 0),
                                             stop=(k == KB - 1))
                for r in range(CH):
                    nc.scalar.copy(s_sb[:, r], s_ps[r][:])
                return s_sb

            def sec_m23(b, V, w_sb):
                """M23: du = u @ W + G, stored out."""
                du_sb = outsp.tile([128, KB, P], BF16, tag="du", name=f"du_{b}")
                for r in range(CH):
                    ps = psum.tile([128, P], F32, tag="ps", name=f"du{b}_{r}")
                    if "UT8pair" in V:
                        for j in range(2):
                            nc.tensor.matmul(ps[:], V["UT8pair"](j, r),
                                             w_sb[:, 2 * j:2 * j + 2],
                                             perf_mode=DR, start=(j == 0),
                                             stop=(j == 1))
                    else:
                        for k in range(KB):
                            nc.tensor.matmul(ps[:],
                                             V["UT"](k)[:, r * 128:(r + 1) * 128],
                                             w_sb[:, k], start=(k == 0),
                                             stop=(k == KB - 1))
                    if "UT8pair" in V:
                        nc.vector.scalar_tensor_tensor(du_sb[:, r], ps[:],
                                                       4.0, V["Gn"](r)[:],
                                                       AOP.mult, AOP.add)
                    else:
                        nc.vector.tensor_tensor(du_sb[:, r], ps[:],
                                                V["Gn"](r)[:], AOP.add)
                    if b == BLOC - 1:
                        nc.scalar.dma_start(d_du[b][:, r], du_sb[:, r])
                if b < BLOC - 1:
                    nc.scalar.dma_start(d_du[b], du_sb[:])

            def sec_tail(b, V, s_sb):
                """C = S + S^T, then dlam = lam @ A (fp8 DR) + C @ u."""
                coup_sb = midp.tile([128, KB, N], BF16, tag="coup",
                                    name=f"coup_{b}")
                for r in range(CH):
                    tps = psum.tile([128, N], BF16, tag="tps", bufs=2,
                                    name=f"tps{b}_{r}")
                    for c in range(KB):
                        nc.tensor.transpose(tps[:, c * 128:(c + 1) * 128],
                                            s_sb[:, c, r * 128:(r + 1) * 128],
                                            identb[:])
                    nc.vector.tensor_tensor(coup_sb[:, r], tps[:], s_sb[:, r],
                                            AOP.add)

                dlam_sb = outsp.tile([128, KB, P], BF16, tag="dlam",
                                     name=f"dlam_{b}")
                dlam_ps = [psum.tile([128, P], F32, tag="ps", name=f"dl{b}_{r}")
                           for r in range(CH)]
                for r in range(CH):
                    for j in range(2):
                        nc.tensor.matmul(dlam_ps[r][:], V["L8pair"](j, r),
                                         V["A8pair"](j), perf_mode=DR,
                                         start=(j == 0), stop=False,
                                         skip_group_check=True)
                for r in range(CH):
                    ps = dlam_ps[r]
                    for k in range(KB):
                        nc.tensor.matmul(ps[:],
                                         coup_sb[:, k, r * 128:(r + 1) * 128],
                                         V["Un"](k)[:], start=False,
                                         stop=(k == KB - 1),
                                         skip_group_check=True)
                    # alternate copy engines so the copies drain in parallel;
                    # the LAST chunk's copy goes to the (faster) vector engine
                    # so the final store issues as early as possible
                    if r % 2 == 0:
                        nc.scalar.copy(dlam_sb[:, r], ps[:])
                    else:
                        nc.vector.tensor_copy(dlam_sb[:, r], ps[:])
                    if b == BLOC - 1:
                        if r < CH - 1:
                            qq = nc.sync if r % 2 == 0 else nc.scalar
                            qq.dma_start(d_dlam[b][:, r], dlam_sb[:, r])
                        else:
                            # split the very last store across both HWDGE
                            # rings so its halves drain in parallel and the
                            # end-of-kernel wait sees a 64KB receipt, not 128KB
                            nc.sync.dma_start(d_dlam[b][:, r, 0:256],
                                              dlam_sb[:, r, 0:256])
                            nc.scalar.dma_start(d_dlam[b][:, r, 256:512],
                                                dlam_sb[:, r, 256:512])
                if b < BLOC - 1:
                    nc.scalar.dma_start(d_dlam[b], dlam_sb[:])

            # all batches run SOLO: within one batch the section chain
            # M1 -> M5 -> M23 -> tail already overlaps every cross-engine
            # handoff (W's DVE pass runs under M5, the S copies under M23,
            # the coupling add under the tail's DR matmuls), and solo
            # sequencing needs each batch's inputs ~5us later than pairing —
            # decisive, because the head DMA stream measures fully saturated
            # (~357 GB/s) and the PE otherwise catches up with it around b3.
            for group in [(b,) for b in range(BLOC)]:
                Vs = [mk_views(b) for b in group]
                ws = [sec_m1(b, V) for b, V in zip(group, Vs)]
                ss = [sec_m5(b, V) for b, V in zip(group, Vs)]
                for b, V, w in zip(group, Vs, ws):
                    sec_m23(b, V, w)
                for b, V, s in zip(group, Vs, ss):
                    sec_tail(b, V, s)

    nc.compile()
    return nc


_NC = None


def _pack(x, dt):
    """[BLOC,512,512] -> [BLOC,128,4,512] in SBUF layout (partition-major)."""
    return np.ascontiguousarray(
        x.reshape(BLOC, KB, 128, P).transpose(0, 2, 1, 3).astype(dt))


def _unpack(y):
    """[BLOC,128,4,512] bf16 -> [BLOC,512,512] fp32."""
    return y.transpose(0, 2, 1, 3).reshape(BLOC, N, P).astype(np.float32)


def _make_in_maps(u, lam, A, G):
    u = np.asarray(u, dtype=np.float32)
    lam = np.asarray(lam, dtype=np.float32)
    A = np.asarray(A, dtype=np.float32)
    G = np.asarray(G, dtype=np.float32)

    in_maps = []
    for c in range(NCORES):
        sl = slice(c * BLOC, (c + 1) * BLOC)
        uc, lamc, Ac, Gc = u[sl], lam[sl], A[sl], G[sl]
        At = np.swapaxes(Ac, 1, 2)
        lamt = np.swapaxes(lamc, 1, 2)
        in1 = np.empty((BLOC, 128, 2 * KB, P), dtype=NP_BF16)
        in1[:, :, 0::2] = _pack(uc, NP_BF16)
        in1[:, :, 1::2] = _pack(Gc, NP_BF16)
        in2 = np.empty((BLOC, 128, 3 * KB, P), dtype=NP_BF16)
        in2[:, :, 0:2 * KB:2] = _pack(lamt, NP_BF16)
        in2[:, :, 1:2 * KB:2] = _pack(np.swapaxes(Gc, 1, 2), NP_BF16)
        in2[:, :, 2 * KB:] = _pack(np.swapaxes(uc, 1, 2), NP_BF16)
        # lamt8 is quantized from the bf16 lamt (same value chain as on-device)
        in8 = np.concatenate([_pack(Ac - At, NP_F8), _pack(Ac, NP_F8),
                              _pack(lamt, NP_BF16).astype(NP_F8)], axis=2)
        # batch-1 fp8 DoubleRow operands, quantized from the same bf16 chain:
        # [u8/4 | g8 | gt8 | ut8] — u8 quantized then scaled by 1/4 (an
        # exact exponent shift) so the on-device W/4 fits fp8e4's range
        u8q = in1[1][:, 0::2].astype(NP_F8).astype(np.float32) / 4.0
        x8b1 = np.ascontiguousarray(np.concatenate(
            [u8q, in1[1][:, 1::2].astype(np.float32),
             in2[1][:, 1:2 * KB:2].astype(np.float32),
             in2[1][:, 2 * KB:].astype(np.float32)], axis=1)).astype(NP_F8)
        # h1f8: [u0,g0 | u1,g1 | u2,u3,g2,g3]; h2f8: [gt0..3 | ut0..3]
        h1f8 = np.ascontiguousarray(
            in1[0][:, [0, 1, 2, 3, 4, 6, 5, 7]]).astype(NP_F8)
        h2f8 = np.ascontiguousarray(
            in2[0][:, [1, 3, 5, 7, 8, 9, 10, 11]]).astype(NP_F8)
        in_maps.append({"in1": in1, "in2": in2, "in8": in8, "x8b1": x8b1,
                        "h2f8": h2f8, "h1f8": h1f8})
    return in_maps


def kernel(u, lam, A, G, t=None, **_ignored):
    global _NC
    if _NC is None:
        _NC = _build_nc()
    nc = _NC

    in_maps = _make_in_maps(u, lam, A, G)
    res = run_bass_kernel_spmd(nc, in_maps, list(range(NCORES)))
    du = np.concatenate([_unpack(res.results[c]["du"]) for c in range(NCORES)],
                        axis=0)
    dlam = np.concatenate([_unpack(res.results[c]["dlam"])
                           for c in range(NCORES)], axis=0)
    return du, dlam



# revision 42
# speedup vs baseline: 1.0011x; 1.0011x over previous
"""Trainium2 Bass kernel for nn_AugmentedODE (B=64, N=P=512), 8-core data parallel.

Per batch the reference computes (7 matmuls of 512^3):
    Omega   = 0.5*(A - A^T)
    du      = u @ Omega + G - u @ (u^T G)
    S       = lam @ G^T
    dlam    = lam @ A + (S + S^T) @ u

Restructured to 5 matmuls + 1 PE transpose set per batch:
    UTG = u^T G                      (bf16:  lhsT=u,    rhs=G)
    W   = 0.5*(A - A^T) - UTG        (DVE; A - A^T precomputed host-side, fp8)
    du  = u @ W + G                  (bf16:  lhsT=u^T,  rhs=W; +G fused in DVE)
    S   = lam @ G^T                  (bf16:  lhsT=lam^T, rhs=G^T)
    C   = S + S^T                    (bf16 PE transpose + DVE add)
    dlam= lam @ A + C @ u            (lam@A in fp8 DoubleRow at 2x rate; C@u bf16;
                                      both accumulated into one PSUM group)

Rel-err budget is 2e-2 (Frobenius); measured 1.734e-2 for this mix (du
1.734e-2, dlam 1.689e-2 — deterministic, seed-0 inputs).  The four
magnitude-dominant matmuls (UTG, u@W, S, C@u) stay bf16 for batches 2-7 and
run at the PE's bf16 roofline (1 row/cycle, ~216 ns per 128x128x512 matmul).
Batch 0 is fully fp8 (halves its head DMA) and runs its S matmul + half its
UTG in fp8 DoubleRow (numerically identical to its plain-fp8 form, ~2.4us
PE saved).  Batch 1 runs UTG, S AND u@W in fp8 DoubleRow (~4.8us PE saved);
u8 is shipped pre-scaled by 1/4 (exact) so the on-device W/4 fits fp8e4's
+-240 range (|W| itself reaches ~590 — product-of-Gaussians tails), and the
du add rescales by 4.  lam@A (~3% of |dlam|) and the skew term (~5% of |W|)
are fp8 DR everywhere.  The error budget is now ~87% consumed; converting
any further bf16 matmul instance to fp8 (+~1e-4 variance each) would cross
the 2e-2 gate.

(Note: offloading the S+S^T transposes to the DMA xbar was tried and
reverted — Tile globally serializes dma-transposes against ALL in-flight
HWDGE DMAs as a HW-deadlock guard, which strings them out by 15-25us and
starves the PE.  The 16 PE transposes per batch stream at ~56ns each and
are the cheapest correct option.)

All operands are pre-packed on the host into the exact SBUF layout
([128 partitions, kblock, 512] with k-blocks contiguous per partition) and
concatenated into three blobs per batch, so every DMA line is multi-KB
contiguous on both sides.  Dependency tracking is per-tile, so batch 0 uses
peeled per-piece tiles whose DMAs are sequenced on one queue in consumption
order; batches 1-3 stream whole blobs on the same queue behind them; later
batches prefetch on parallel queues, gated naturally by the 4-deep input
rings.  Batches are processed in PAIRS with sections interleaved
(M1(b), M1(b+1), M5(b), M5(b+1), ...) so every section boundary is followed
by independent work and cross-engine handoff latencies stay off the PE
critical path.  Outputs are written bf16 and upcast on the host.
"""
import numpy as np
import ml_dtypes

import concourse.bass as bass
import concourse.mybir as mybir
import concourse.tile as tile
from concourse import bacc
from concourse.bass_utils import run_bass_kernel_spmd
from concourse.masks import make_identity

F32 = mybir.dt.float32
F32R = mybir.dt.float32r
BF16 = mybir.dt.bfloat16
F8 = mybir.dt.float8e4
AOP = mybir.AluOpType
DR = mybir.MatmulPerfMode.DoubleRow

NP_BF16 = ml_dtypes.bfloat16
NP_F8 = ml_dtypes.float8_e4m3

B, N, P = 64, 512, 512
NCORES = 8
BLOC = B // NCORES          # batches per core
KB = 4                      # 512 = 4 k-blocks of 128
CH = 4                      # 4 output chunks of 128 rows
WARMUP_MM = 6


def _build_nc():
    nc = bacc.Bacc("TRN2", target_bir_lowering=False, debug=False,
                   num_devices=NCORES)

    # in1: interleaved [u0,g0,u1,g1,u2,g2,u3,g3]
    # in2: interleaved [lamt0,gt0,...,lamt3,gt3, ut0..ut3]
    # in8: amat=A-A^T(0:4) | a(4:8) | lamt8(8:12), fp8e4
    d_in1 = nc.declare_dram_parameter("in1", [BLOC, 128, 2 * KB, P], BF16,
                                      isOutput=False)
    d_in2 = nc.declare_dram_parameter("in2", [BLOC, 128, 3 * KB, P], BF16,
                                      isOutput=False)
    d_in8 = nc.declare_dram_parameter("in8", [BLOC, 128, 3 * KB, P], F8,
                                      isOutput=False)
    # batch-0 fp8 head blobs: h2f8 = [gt0..3 | ut0..3] (its lamt fp8 comes
    # from in8's lamt8 blocks, so it is not shipped twice), h1f8 =
    # [u0,g0 | u1,g1 | u2,u3,g2,g3] (last four blocks reordered so the
    # (u2,u3)x(g2,g3) k-pair runs as one fp8 DoubleRow pass)
    d_h2f8 = nc.declare_dram_parameter("h2f8", [128, 2 * KB, P], F8,
                                       isOutput=False)
    d_h1f8 = nc.declare_dram_parameter("h1f8", [128, 2 * KB, P], F8,
                                       isOutput=False)
    # batch-1 fp8 operands for its DoubleRow UTG, S and u@W matmuls:
    # [u8(0:4) | g8(4:8) | gt8(8:12) | ut8(12:16)]
    d_x8b1 = nc.declare_dram_parameter("x8b1", [128, 4 * KB, P], F8,
                                       isOutput=False)
    d_du = nc.declare_dram_parameter("du", [BLOC, 128, KB, P], BF16,
                                     isOutput=True)
    d_dlam = nc.declare_dram_parameter("dlam", [BLOC, 128, KB, P], BF16,
                                       isOutput=True)

    with tile.TileContext(nc) as tc:
        with (
            tc.tile_pool(name="const", bufs=1) as constp,
            tc.tile_pool(name="ins", bufs=4) as insp,
            tc.tile_pool(name="mid", bufs=2) as midp,
            tc.tile_pool(name="outs", bufs=2) as outsp,
            tc.tile_pool(name="psum", bufs=6, space="PSUM") as psum,
        ):
            # warm-up source first: one memset, no identity dependency, so the
            # PE's ~6us low-clock window starts at the earliest possible point
            # memset on DVE: gpsimd is busy with its framework preamble
            # (DMA rings, library loads) until ~7us, which would delay the
            # first warm-up matmul and with it the HAM un-throttle point
            wsrc = constp.tile([128, 512], BF16)
            nc.vector.memset(wsrc[:].bitcast(F32), 0.0)
            warm_ps = psum.tile([128, 512], F32, tag="ps")
            for i in range(WARMUP_MM):
                nc.tensor.matmul(warm_ps[:], wsrc[:, 0:128], wsrc[:],
                                 start=True, stop=True)

            identf = constp.tile([128, 128], F32)
            make_identity(nc, identf[:])
            identb = constp.tile([128, 128], BF16)
            nc.gpsimd.tensor_copy(identb[:], identf[:])

            # batch-0 peeled tiles: per-piece DMAs on ONE queue in consumption
            # order (dep tracking is per-tile, so fine pieces start compute early)
            h1a = insp.tile([128, 2, P], F8, tag="h1a", bufs=1)        # u0,g0
            h1b1 = insp.tile([128, 2, P], F8, tag="h1b1", bufs=1)      # u1,g1
            h1b2 = insp.tile([128, 4, P], F8, tag="h1b2", bufs=1)      # u2,u3,g2,g3
            h8a = insp.tile([128, KB, P], F8, tag="h8a", bufs=1)       # amat
            hgt = insp.tile([128, KB, P], F8, tag="hgt", bufs=1)       # gt0..3
            h8b2 = insp.tile([128, KB, P], F8, tag="h8b2", bufs=1)     # lamt8
            h2c = insp.tile([128, KB, P], F8, tag="h2c", bufs=1)       # ut
            h8b1 = insp.tile([128, KB, P], F8, tag="h8b1", bufs=1)     # a
            nc.sync.dma_start(h1a[:], d_h1f8[:, 0:2])
            nc.sync.dma_start(h1b1[:], d_h1f8[:, 2:4])
            nc.sync.dma_start(h1b2[:], d_h1f8[:, 4:2 * KB])
            nc.sync.dma_start(h8a[:], d_in8[0][:, 0:KB])
            nc.sync.dma_start(hgt[:], d_h2f8[:, 0:KB])
            nc.sync.dma_start(h8b2[:], d_in8[0][:, 2 * KB:3 * KB])
            nc.sync.dma_start(h2c[:], d_h2f8[:, KB:2 * KB])
            nc.sync.dma_start(h8b1[:], d_in8[0][:, KB:2 * KB])

            def mk_views(b):
                """Allocate input tiles (b>0), emit their DMAs, return accessors."""
                if b == 0:
                    h1 = [h1a, h1b1, h1b2, h1b2]
                    h1o = [0, 0, 0, 1]
                    g1o = [1, 1, 2, 3]
                    return dict(
                        Un=lambda k: h1[k][:, h1o[k]],
                        Gn=lambda k: h1[k][:, g1o[k]],
                        UT=lambda k: h2c[:, k],
                        AM8=lambda k: h8a[:, k],
                        A8pair=lambda j: h8b1[:, 2 * j:2 * j + 2],
                        L8pair=lambda j, r: h8b2[:, 2 * j:2 * j + 2,
                                                 r * 128:(r + 1) * 128],
                        GT8pair=lambda j: hgt[:, 2 * j:2 * j + 2],
                        U8H=lambda r: h1b2[:, 0:2, r * 128:(r + 1) * 128],
                        G8H=lambda: h1b2[:, 2:4],
                    )
                if b == 1:
                    # batch 1 computes UTG, S, AND u@W in fp8 DoubleRow from
                    # x8b1 (= [u8 | g8 | gt8 | ut8]); its bf16 lamt/gt/ut are
                    # never read, nor is its bf16 G (the du "+G" add reads
                    # g8 — G is ~1/512 of |du|, so fp8 G costs nothing).
                    # Only the u half of in1[1] is still loaded in bf16 (the
                    # C@u right operand, which feeds the dominant dlam term).
                    # (Peeling these blobs into finer per-section pieces was
                    # tried and reverted: the head stream is bandwidth-bound,
                    # so pieces only redistribute the same wait while adding
                    # per-piece completion-latency jitter.)
                    in8 = insp.tile([128, 3 * KB, P], F8, tag="in8",
                                    name="in8_1")
                    x8 = insp.tile([128, 4 * KB, P], F8, tag="x8b1", bufs=1)
                    in1u = insp.tile([128, KB, P], BF16, tag="in1b1", bufs=1)
                    nc.sync.dma_start(x8[:], d_x8b1[:])
                    nc.sync.dma_start(in8[:], d_in8[1])
                    nc.sync.dma_start(in1u[:], d_in1[1][:, 0:2 * KB:2])
                    return dict(
                        Un=lambda k, t=in1u: t[:, k],
                        Gn=lambda k, t=x8: t[:, KB + k],
                        AM8=lambda k, t=in8: t[:, k],
                        A8pair=lambda j, t=in8: t[:, KB + 2 * j:KB + 2 * j + 2],
                        L8pair=lambda j, r, t=in8: t[:, 2 * KB + 2 * j:
                                                     2 * KB + 2 * j + 2,
                                                     r * 128:(r + 1) * 128],
                        U8pair=lambda j, r, t=x8: t[:, 2 * j:2 * j + 2,
                                                    r * 128:(r + 1) * 128],
                        G8pair=lambda j, t=x8: t[:, KB + 2 * j:KB + 2 * j + 2],
                        GT8pair=lambda j, t=x8: t[:, 2 * KB + 2 * j:
                                                  2 * KB + 2 * j + 2],
                        UT8pair=lambda j, r, t=x8: t[:, 3 * KB + 2 * j:
                                                     3 * KB + 2 * j + 2,
                                                     r * 128:(r + 1) * 128],
                    )
                # bufs=3: only b2..b7 allocate from these rings (b1 uses
                # private slim tiles), so 3 slots restore the "b5 gated on
                # b2's release" back-pressure that keeps late batches' loads
                # from stealing head DMA bandwidth.
                in1 = insp.tile([128, 2 * KB, P], BF16, tag="in1", bufs=3,
                                name=f"in1_{b}")
                in2 = insp.tile([128, 3 * KB, P], BF16, tag="in2", bufs=3,
                                name=f"in2_{b}")
                in8 = insp.tile([128, 3 * KB, P], F8, tag="in8",
                                name=f"in8_{b}")
                if b <= 4:
                    # head batches: ordered on sync behind batch 0's pieces,
                    # so nothing dilutes the stream batch 0/1 are waiting on
                    nc.sync.dma_start(in1[:], d_in1[b])
                    nc.sync.dma_start(in8[:], d_in8[b])
                    nc.sync.dma_start(in2[:], d_in2[b])
                else:
                    # b>=5 are ring-gated behind live tiles (slot frees only
                    # after batch b-4 completes), so parallel queues can't
                    # steal head bandwidth
                    nc.sync.dma_start(in1[:], d_in1[b])
                    nc.gpsimd.dma_start(in2[:], d_in2[b])
                    nc.gpsimd.dma_start(in8[:], d_in8[b])
                return dict(
                    Un=lambda k, t=in1: t[:, 2 * k],
                    Gn=lambda k, t=in1: t[:, 2 * k + 1],
                    LT=lambda k, t=in2: t[:, 2 * k],
                    GT=lambda k, t=in2: t[:, 2 * k + 1],
                    UT=lambda k, t=in2: t[:, 2 * KB + k],
                    AM8=lambda k, t=in8: t[:, k],
                    A8pair=lambda j, t=in8: t[:, KB + 2 * j:KB + 2 * j + 2],
                    L8pair=lambda j, r, t=in8: t[:, 2 * KB + 2 * j:
                                                 2 * KB + 2 * j + 2,
                                                 r * 128:(r + 1) * 128],
                )

            def sec_m1(b, V):
                """M1: UTG = u^T G (k-outer) ; W = 0.5*amat - UTG (DVE)."""
                if "UT8pair" in V:
                    # batch 1: W is written in fp8 so u@W can run DoubleRow
                    # (W's k-blocks are pair-adjacent in this layout)
                    w_sb = midp.tile([128, KB, P], F8, tag="w8", bufs=1,
                                     name=f"w8_{b}")
                else:
                    w_sb = midp.tile([128, KB, P], BF16, tag="w",
                                     name=f"w_{b}")
                utg = [psum.tile([128, P], F32, tag="ps", name=f"utg{b}_{r}")
                       for r in range(CH)]
                if "U8pair" in V:
                    for j in range(2):
                        for r in range(CH):
                            nc.tensor.matmul(utg[r][:], V["U8pair"](j, r),
                                             V["G8pair"](j), perf_mode=DR,
                                             start=(j == 0), stop=(j == 1))
                elif "U8H" in V:
                    # batch 0 hybrid: k=0,1 as plain fp8 passes (each gated on
                    # a 128KB head piece, so compute starts at first-piece
                    # arrival), then the (u2,u3) k-pair as one DR pass
                    for k in range(2):
                        for r in range(CH):
                            nc.tensor.matmul(utg[r][:],
                                             V["Un"](k)[:, r * 128:(r + 1) * 128],
                                             V["Gn"](k)[:], start=(k == 0),
                                             stop=False)
                    for r in range(CH):
                        nc.tensor.matmul(utg[r][:], V["U8H"](r), V["G8H"](),
                                         perf_mode=DR, start=False, stop=True,
                                         skip_group_check=True)
                else:
                    for k in range(KB):
                        for r in range(CH):
                            nc.tensor.matmul(utg[r][:],
                                             V["Un"](k)[:, r * 128:(r + 1) * 128],
                                             V["Gn"](k)[:], start=(k == 0),
                                             stop=(k == KB - 1))
                # batch 1: u8 is shipped pre-scaled by 1/4 (exact exponent
                # shift), so utg = UTG/4 and W8 = W/4 stays inside fp8e4's
                # +-240 range (|W| itself reaches ~590); the du add scales
                # the matmul result back by 4.
                wscale = 0.125 if "UT8pair" in V else 0.5
                for r in range(CH):
                    nc.vector.scalar_tensor_tensor(w_sb[:, r], V["AM8"](r)[:],
                                                   wscale, utg[r][:], AOP.mult,
                                                   AOP.subtract)
                return w_sb

            def sec_m5(b, V):
                """M5: S = lam @ G^T (k-outer)."""
                s_sb = midp.tile([128, KB, N], BF16, tag="s", name=f"s_{b}")
                s_ps = [psum.tile([128, N], F32, tag="ps", name=f"s{b}_{r}")
                        for r in range(CH)]
                if "GT8pair" in V:
                    for j in range(2):
                        for r in range(CH):
                            nc.tensor.matmul(s_ps[r][:], V["L8pair"](j, r),
                                             V["GT8pair"](j), perf_mode=DR,
                                             start=(j == 0), stop=(j == 1))
                else:
                    for k in range(KB):
                        for r in range(CH):
                            nc.tensor.matmul(s_ps[r][:],
                                             V["LT"](k)[:, r * 128:(r + 1) * 128],
                                             V["GT"](k)[:], start=(k == 0),
                                             stop=(k == KB - 1))
                for r in range(CH):
                    nc.scalar.copy(s_sb[:, r], s_ps[r][:])
                return s_sb

            def sec_m23(b, V, w_sb):
                """M23: du = u @ W + G, stored out."""
                du_sb = outsp.tile([128, KB, P], BF16, tag="du", name=f"du_{b}")
                for r in range(CH):
                    ps = psum.tile([128, P], F32, tag="ps", name=f"du{b}_{r}")
                    if "UT8pair" in V:
                        for j in range(2):
                            nc.tensor.matmul(ps[:], V["UT8pair"](j, r),
                                             w_sb[:, 2 * j:2 * j + 2],
                                             perf_mode=DR, start=(j == 0),
                                             stop=(j == 1))
                    else:
                        for k in range(KB):
                            nc.tensor.matmul(ps[:],
                                             V["UT"](k)[:, r * 128:(r + 1) * 128],
                                             w_sb[:, k], start=(k == 0),
                                             stop=(k == KB - 1))
                    if "UT8pair" in V:
                        nc.vector.scalar_tensor_tensor(du_sb[:, r], ps[:],
                                                       4.0, V["Gn"](r)[:],
                                                       AOP.mult, AOP.add)
                    else:
                        nc.vector.tensor_tensor(du_sb[:, r], ps[:],
                                                V["Gn"](r)[:], AOP.add)
                    if b == BLOC - 1:
                        nc.scalar.dma_start(d_du[b][:, r], du_sb[:, r])
                if b < BLOC - 1:
                    nc.scalar.dma_start(d_du[b], du_sb[:])

            def sec_tail(b, V, s_sb):
                """C = S + S^T, then dlam = lam @ A (fp8 DR) + C @ u."""
                coup_sb = midp.tile([128, KB, N], BF16, tag="coup",
                                    name=f"coup_{b}")
                for r in range(CH):
                    tps = psum.tile([128, N], BF16, tag="tps", bufs=2,
                                    name=f"tps{b}_{r}")
                    for c in range(KB):
                        nc.tensor.transpose(tps[:, c * 128:(c + 1) * 128],
                                            s_sb[:, c, r * 128:(r + 1) * 128],
                                            identb[:])
                    nc.vector.tensor_tensor(coup_sb[:, r], tps[:], s_sb[:, r],
                                            AOP.add)

                dlam_sb = outsp.tile([128, KB, P], BF16, tag="dlam",
                                     name=f"dlam_{b}")
                dlam_ps = [psum.tile([128, P], F32, tag="ps", name=f"dl{b}_{r}")
                           for r in range(CH)]
                for r in range(CH):
                    for j in range(2):
                        nc.tensor.matmul(dlam_ps[r][:], V["L8pair"](j, r),
                                         V["A8pair"](j), perf_mode=DR,
                                         start=(j == 0), stop=False,
                                         skip_group_check=True)
                for r in range(CH):
                    ps = dlam_ps[r]
                    for k in range(KB):
                        nc.tensor.matmul(ps[:],
                                         coup_sb[:, k, r * 128:(r + 1) * 128],
                                         V["Un"](k)[:], start=False,
                                         stop=(k == KB - 1),
                                         skip_group_check=True)
                    # alternate copy engines so the copies drain in parallel;
                    # the LAST chunk's copy goes to the (faster) vector engine
                    # so the final store issues as early as possible
                    if r % 2 == 0:
                        nc.scalar.copy(dlam_sb[:, r], ps[:])
                    else:
                        nc.vector.tensor_copy(dlam_sb[:, r], ps[:])
                    if b == BLOC - 1:
                        if r < CH - 1:
                            qq = nc.sync if r % 2 == 0 else nc.scalar
                            qq.dma_start(d_dlam[b][:, r], dlam_sb[:, r])
                        else:
                            # split the very last store across both HWDGE
                            # rings so its halves drain in parallel and the
                            # end-of-kernel wait sees a 64KB receipt, not 128KB
                            nc.sync.dma_start(d_dlam[b][:, r, 0:256],
                                              dlam_sb[:, r, 0:256])
                            nc.scalar.dma_start(d_dlam[b][:, r, 256:512],
                                                dlam_sb[:, r, 256:512])
                if b < BLOC - 1:
                    nc.scalar.dma_start(d_dlam[b], dlam_sb[:])

            # all batches run SOLO: within one batch the section chain
            # M1 -> M5 -> M23 -> tail already overlaps every cross-engine
            # handoff (W's DVE pass runs under M5, the S copies under M23,
            # the coupling add under the tail's DR matmuls), and solo
            # sequencing needs each batch's inputs ~5us later than pairing —
            # decisive, because the head DMA stream measures fully saturated
            # (~357 GB/s) and the PE otherwise catches up with it around b3.
            for group in [(b,) for b in range(BLOC)]:
                Vs = [mk_views(b) for b in group]
                ws = [sec_m1(b, V) for b, V in zip(group, Vs)]
                ss = [sec_m5(b, V) for b, V in zip(group, Vs)]
                for b, V, w in zip(group, Vs, ws):
                    sec_m23(b, V, w)
                for b, V, s in zip(group, Vs, ss):
                    sec_tail(b, V, s)

    nc.compile()
    return nc


_NC = None


def _pack(x, dt):
    """[BLOC,512,512] -> [BLOC,128,4,512] in SBUF layout (partition-major)."""
    return np.ascontiguousarray(
        x.reshape(BLOC, KB, 128, P).transpose(0, 2, 1, 3).astype(dt))


def _unpack(y):
    """[BLOC,128,4,512] bf16 -> [BLOC,512,512] fp32."""
    return y.transpose(0, 2, 1, 3).reshape(BLOC, N, P).astype(np.float32)


def _make_in_maps(u, lam, A, G):
    u = np.asarray(u, dtype=np.float32)
    lam = np.asarray(lam, dtype=np.float32)
    A = np.asarray(A, dtype=np.float32)
    G = np.asarray(G, dtype=np.float32)

    in_maps = []
    for c in range(NCORES):
        sl = slice(c * BLOC, (c + 1) * BLOC)
        uc, lamc, Ac, Gc = u[sl], lam[sl], A[sl], G[sl]
        At = np.swapaxes(Ac, 1, 2)
        lamt = np.swapaxes(lamc, 1, 2)
        in1 = np.empty((BLOC, 128, 2 * KB, P), dtype=NP_BF16)
        in1[:, :, 0::2] = _pack(uc, NP_BF16)
        in1[:, :, 1::2] = _pack(Gc, NP_BF16)
        in2 = np.empty((BLOC, 128, 3 * KB, P), dtype=NP_BF16)
        in2[:, :, 0:2 * KB:2] = _pack(lamt, NP_BF16)
        in2[:, :, 1:2 * KB:2] = _pack(np.swapaxes(Gc, 1, 2), NP_BF16)
        in2[:, :, 2 * KB:] = _pack(np.swapaxes(uc, 1, 2), NP_BF16)
        # lamt8 is quantized from the bf16 lamt (same value chain as on-device)
        in8 = np.concatenate([_pack(Ac - At, NP_F8), _pack(Ac, NP_F8),
                              _pack(lamt, NP_BF16).astype(NP_F8)], axis=2)
        # batch-1 fp8 DoubleRow operands, quantized from the same bf16 chain:
        # [u8/4 | g8 | gt8 | ut8] — u8 quantized then scaled by 1/4 (an
        # exact exponent shift) so the on-device W/4 fits fp8e4's range
        u8q = in1[1][:, 0::2].astype(NP_F8).astype(np.float32) / 4.0
        x8b1 = np.ascontiguousarray(np.concatenate(
            [u8q, in1[1][:, 1::2].astype(np.float32),
             in2[1][:, 1:2 * KB:2].astype(np.float32),
             in2[1][:, 2 * KB:].astype(np.float32)], axis=1)).astype(NP_F8)
        # h1f8: [u0,g0 | u1,g1 | u2,u3,g2,g3]; h2f8: [gt0..3 | ut0..3]
        h1f8 = np.ascontiguousarray(
            in1[0][:, [0, 1, 2, 3, 4, 6, 5, 7]]).astype(NP_F8)
        h2f8 = np.ascontiguousarray(
            in2[0][:, [1, 3, 5, 7, 8, 9, 10, 11]]).astype(NP_F8)
        in_maps.append({"in1": in1, "in2": in2, "in8": in8, "x8b1": x8b1,
                        "h2f8": h2f8, "h1f8": h1f8})
    return in_maps


def kernel(u, lam, A, G, t=None, **_ignored):
    global _NC
    if _NC is None:
        _NC = _build_nc()
    nc = _NC

    in_maps = _make_in_maps(u, lam, A, G)
    res = run_bass_kernel_spmd(nc, in_maps, list(range(NCORES)))
    du = np.concatenate([_unpack(res.results[c]["du"]) for c in range(NCORES)],
                        axis=0)
    dlam = np.concatenate([_unpack(res.results[c]["dlam"])
                           for c in range(NCORES)], axis=0)
    return du, dlam



# revision 43
# speedup vs baseline: 1.0058x; 1.0047x over previous
"""Trainium2 Bass kernel for nn_AugmentedODE (B=64, N=P=512), 8-core data parallel.

Per batch the reference computes (7 matmuls of 512^3):
    Omega   = 0.5*(A - A^T)
    du      = u @ Omega + G - u @ (u^T G)
    S       = lam @ G^T
    dlam    = lam @ A + (S + S^T) @ u

Restructured to 5 matmuls + 1 PE transpose set per batch:
    UTG = u^T G                      (bf16:  lhsT=u,    rhs=G)
    W   = 0.5*(A - A^T) - UTG        (DVE; A - A^T precomputed host-side, fp8)
    du  = u @ W + G                  (bf16:  lhsT=u^T,  rhs=W; +G fused in DVE)
    S   = lam @ G^T                  (bf16:  lhsT=lam^T, rhs=G^T)
    C   = S + S^T                    (bf16 PE transpose + DVE add)
    dlam= lam @ A + C @ u            (lam@A in fp8 DoubleRow at 2x rate; C@u bf16;
                                      both accumulated into one PSUM group)

Rel-err budget is 2e-2 (Frobenius); measured 1.734e-2 for this mix (du
1.734e-2, dlam 1.689e-2 — deterministic, seed-0 inputs).  The four
magnitude-dominant matmuls (UTG, u@W, S, C@u) stay bf16 for batches 2-7 and
run at the PE's bf16 roofline (1 row/cycle, ~216 ns per 128x128x512 matmul).
Batch 0 is fully fp8 (halves its head DMA) and runs its S matmul + half its
UTG in fp8 DoubleRow (numerically identical to its plain-fp8 form, ~2.4us
PE saved).  Batch 1 runs UTG, S AND u@W in fp8 DoubleRow (~4.8us PE saved);
u8 is shipped pre-scaled by 1/4 (exact) so the on-device W/4 fits fp8e4's
+-240 range (|W| itself reaches ~590 — product-of-Gaussians tails), and the
du add rescales by 4.  lam@A (~3% of |dlam|) and the skew term (~5% of |W|)
are fp8 DR everywhere.  The error budget is now ~87% consumed; converting
any further bf16 matmul instance to fp8 (+~1e-4 variance each) would cross
the 2e-2 gate.

(Note: offloading the S+S^T transposes to the DMA xbar was tried and
reverted — Tile globally serializes dma-transposes against ALL in-flight
HWDGE DMAs as a HW-deadlock guard, which strings them out by 15-25us and
starves the PE.  The 16 PE transposes per batch stream at ~56ns each and
are the cheapest correct option.)

All operands are pre-packed on the host into the exact SBUF layout
([128 partitions, kblock, 512] with k-blocks contiguous per partition) and
concatenated into three blobs per batch, so every DMA line is multi-KB
contiguous on both sides.  Dependency tracking is per-tile, so batch 0 uses
peeled per-piece tiles whose DMAs are sequenced on one queue in consumption
order; batches 1-3 stream whole blobs on the same queue behind them; later
batches prefetch on parallel queues, gated naturally by the 4-deep input
rings.  Batches are processed in PAIRS with sections interleaved
(M1(b), M1(b+1), M5(b), M5(b+1), ...) so every section boundary is followed
by independent work and cross-engine handoff latencies stay off the PE
critical path.  Outputs are written bf16 and upcast on the host.
"""
import numpy as np
import ml_dtypes

import concourse.bass as bass
import concourse.mybir as mybir
import concourse.tile as tile
from concourse import bacc
from concourse.bass_utils import run_bass_kernel_spmd
from concourse.masks import make_identity

F32 = mybir.dt.float32
F32R = mybir.dt.float32r
BF16 = mybir.dt.bfloat16
F8 = mybir.dt.float8e4
AOP = mybir.AluOpType
DR = mybir.MatmulPerfMode.DoubleRow

NP_BF16 = ml_dtypes.bfloat16
NP_F8 = ml_dtypes.float8_e4m3

B, N, P = 64, 512, 512
NCORES = 8
BLOC = B // NCORES          # batches per core
KB = 4                      # 512 = 4 k-blocks of 128
CH = 4                      # 4 output chunks of 128 rows
WARMUP_MM = 6


def _build_nc():
    nc = bacc.Bacc("TRN2", target_bir_lowering=False, debug=False,
                   num_devices=NCORES)

    # in1: interleaved [u0,g0,u1,g1,u2,g2,u3,g3]
    # in2: interleaved [lamt0,gt0,...,lamt3,gt3, ut0..ut3]
    # in8: amat=A-A^T(0:4) | a(4:8) | lamt8(8:12), fp8e4
    d_in1 = nc.declare_dram_parameter("in1", [BLOC, 128, 2 * KB, P], BF16,
                                      isOutput=False)
    d_in2 = nc.declare_dram_parameter("in2", [BLOC, 128, 3 * KB, P], BF16,
                                      isOutput=False)
    d_in8 = nc.declare_dram_parameter("in8", [BLOC, 128, 3 * KB, P], F8,
                                      isOutput=False)
    # batch-0 fp8 head blobs: h2f8 = [gt0..3 | ut0..3] (its lamt fp8 comes
    # from in8's lamt8 blocks, so it is not shipped twice), h1f8 =
    # [u0,g0 | u1,g1 | u2,u3,g2,g3] (last four blocks reordered so the
    # (u2,u3)x(g2,g3) k-pair runs as one fp8 DoubleRow pass)
    d_h2f8 = nc.declare_dram_parameter("h2f8", [128, 2 * KB, P], F8,
                                       isOutput=False)
    d_h1f8 = nc.declare_dram_parameter("h1f8", [128, 2 * KB, P], F8,
                                       isOutput=False)
    # batch-1 fp8 operands for its DoubleRow UTG, S and u@W matmuls:
    # [u8(0:4) | g8(4:8) | gt8(8:12) | ut8(12:16)]
    d_x8b1 = nc.declare_dram_parameter("x8b1", [128, 4 * KB, P], F8,
                                       isOutput=False)
    d_du = nc.declare_dram_parameter("du", [BLOC, 128, KB, P], BF16,
                                     isOutput=True)
    d_dlam = nc.declare_dram_parameter("dlam", [BLOC, 128, KB, P], BF16,
                                       isOutput=True)

    with tile.TileContext(nc) as tc:
        with (
            tc.tile_pool(name="const", bufs=1) as constp,
            tc.tile_pool(name="ins", bufs=4) as insp,
            tc.tile_pool(name="mid", bufs=2) as midp,
            tc.tile_pool(name="outs", bufs=2) as outsp,
            tc.tile_pool(name="psum", bufs=6, space="PSUM") as psum,
        ):
            # warm-up source first: one memset, no identity dependency, so the
            # PE's ~6us low-clock window starts at the earliest possible point
            # memset on DVE: gpsimd is busy with its framework preamble
            # (DMA rings, library loads) until ~7us, which would delay the
            # first warm-up matmul and with it the HAM un-throttle point
            wsrc = constp.tile([128, 512], BF16)
            nc.vector.memset(wsrc[:].bitcast(F32), 0.0)
            warm_ps = psum.tile([128, 512], F32, tag="ps")
            for i in range(WARMUP_MM):
                nc.tensor.matmul(warm_ps[:], wsrc[:, 0:128], wsrc[:],
                                 start=True, stop=True)

            identf = constp.tile([128, 128], F32)
            make_identity(nc, identf[:])
            identb = constp.tile([128, 128], BF16)
            nc.gpsimd.tensor_copy(identb[:], identf[:])

            # batch-0 peeled tiles: per-piece DMAs on ONE queue in consumption
            # order (dep tracking is per-tile, so fine pieces start compute early)
            h1a = insp.tile([128, 2, P], F8, tag="h1a", bufs=1)        # u0,g0
            h1b1 = insp.tile([128, 2, P], F8, tag="h1b1", bufs=1)      # u1,g1
            h1b2 = insp.tile([128, 4, P], F8, tag="h1b2", bufs=1)      # u2,u3,g2,g3
            h8a = insp.tile([128, KB, P], F8, tag="h8a", bufs=1)       # amat
            hgt = insp.tile([128, KB, P], F8, tag="hgt", bufs=1)       # gt0..3
            h8b2 = insp.tile([128, KB, P], F8, tag="h8b2", bufs=1)     # lamt8
            h2c = insp.tile([128, KB, P], F8, tag="h2c", bufs=1)       # ut
            h8b1 = insp.tile([128, KB, P], F8, tag="h8b1", bufs=1)     # a
            # consumption order: M1's u/g pieces, M5-DR's gt8+lamt8, only
            # THEN the W skew term (the DVE STT runs after M1's PSUM drains
            # anyway), M23's ut8, and the tail's a-blocks last.  Putting
            # amat before gt8/lamt8 was measured as a 0.8-2.3us PE stall:
            # M5(b0) sat waiting while bytes it didn't need yet streamed.
            nc.sync.dma_start(h1a[:], d_h1f8[:, 0:2])
            nc.sync.dma_start(h1b1[:], d_h1f8[:, 2:4])
            nc.sync.dma_start(h1b2[:], d_h1f8[:, 4:2 * KB])
            nc.sync.dma_start(hgt[:], d_h2f8[:, 0:KB])
            nc.sync.dma_start(h8b2[:], d_in8[0][:, 2 * KB:3 * KB])
            nc.sync.dma_start(h8a[:], d_in8[0][:, 0:KB])
            nc.sync.dma_start(h2c[:], d_h2f8[:, KB:2 * KB])
            nc.sync.dma_start(h8b1[:], d_in8[0][:, KB:2 * KB])

            def mk_views(b):
                """Allocate input tiles (b>0), emit their DMAs, return accessors."""
                if b == 0:
                    h1 = [h1a, h1b1, h1b2, h1b2]
                    h1o = [0, 0, 0, 1]
                    g1o = [1, 1, 2, 3]
                    return dict(
                        Un=lambda k: h1[k][:, h1o[k]],
                        Gn=lambda k: h1[k][:, g1o[k]],
                        UT=lambda k: h2c[:, k],
                        AM8=lambda k: h8a[:, k],
                        A8pair=lambda j: h8b1[:, 2 * j:2 * j + 2],
                        L8pair=lambda j, r: h8b2[:, 2 * j:2 * j + 2,
                                                 r * 128:(r + 1) * 128],
                        GT8pair=lambda j: hgt[:, 2 * j:2 * j + 2],
                        U8H=lambda r: h1b2[:, 0:2, r * 128:(r + 1) * 128],
                        G8H=lambda: h1b2[:, 2:4],
                    )
                if b == 1:
                    # batch 1 computes UTG, S, AND u@W in fp8 DoubleRow from
                    # x8b1 (= [u8 | g8 | gt8 | ut8]); its bf16 lamt/gt/ut are
                    # never read, nor is its bf16 G (the du "+G" add reads
                    # g8 — G is ~1/512 of |du|, so fp8 G costs nothing).
                    # Only the u half of in1[1] is still loaded in bf16 (the
                    # C@u right operand, which feeds the dominant dlam term).
                    # (Peeling these blobs into finer per-section pieces was
                    # tried and reverted: the head stream is bandwidth-bound,
                    # so pieces only redistribute the same wait while adding
                    # per-piece completion-latency jitter.)
                    in8 = insp.tile([128, 3 * KB, P], F8, tag="in8",
                                    name="in8_1")
                    x8 = insp.tile([128, 4 * KB, P], F8, tag="x8b1", bufs=1)
                    in1u = insp.tile([128, KB, P], BF16, tag="in1b1", bufs=1)
                    nc.sync.dma_start(x8[:], d_x8b1[:])
                    nc.sync.dma_start(in8[:], d_in8[1])
                    nc.sync.dma_start(in1u[:], d_in1[1][:, 0:2 * KB:2])
                    return dict(
                        Un=lambda k, t=in1u: t[:, k],
                        Gn=lambda k, t=x8: t[:, KB + k],
                        AM8=lambda k, t=in8: t[:, k],
                        A8pair=lambda j, t=in8: t[:, KB + 2 * j:KB + 2 * j + 2],
                        L8pair=lambda j, r, t=in8: t[:, 2 * KB + 2 * j:
                                                     2 * KB + 2 * j + 2,
                                                     r * 128:(r + 1) * 128],
                        U8pair=lambda j, r, t=x8: t[:, 2 * j:2 * j + 2,
                                                    r * 128:(r + 1) * 128],
                        G8pair=lambda j, t=x8: t[:, KB + 2 * j:KB + 2 * j + 2],
                        GT8pair=lambda j, t=x8: t[:, 2 * KB + 2 * j:
                                                  2 * KB + 2 * j + 2],
                        UT8pair=lambda j, r, t=x8: t[:, 3 * KB + 2 * j:
                                                     3 * KB + 2 * j + 2,
                                                     r * 128:(r + 1) * 128],
                    )
                # bufs=3: only b2..b7 allocate from these rings (b1 uses
                # private slim tiles), so 3 slots restore the "b5 gated on
                # b2's release" back-pressure that keeps late batches' loads
                # from stealing head DMA bandwidth.
                in1 = insp.tile([128, 2 * KB, P], BF16, tag="in1", bufs=3,
                                name=f"in1_{b}")
                in2 = insp.tile([128, 3 * KB, P], BF16, tag="in2", bufs=3,
                                name=f"in2_{b}")
                in8 = insp.tile([128, 3 * KB, P], F8, tag="in8",
                                name=f"in8_{b}")
                if b <= 4:
                    # head batches: ordered on sync behind batch 0's pieces,
                    # so nothing dilutes the stream batch 0/1 are waiting on
                    nc.sync.dma_start(in1[:], d_in1[b])
                    nc.sync.dma_start(in8[:], d_in8[b])
                    nc.sync.dma_start(in2[:], d_in2[b])
                else:
                    # b>=5 are ring-gated behind live tiles (slot frees only
                    # after batch b-4 completes), so parallel queues can't
                    # steal head bandwidth
                    nc.sync.dma_start(in1[:], d_in1[b])
                    nc.gpsimd.dma_start(in2[:], d_in2[b])
                    nc.gpsimd.dma_start(in8[:], d_in8[b])
                return dict(
                    Un=lambda k, t=in1: t[:, 2 * k],
                    Gn=lambda k, t=in1: t[:, 2 * k + 1],
                    LT=lambda k, t=in2: t[:, 2 * k],
                    GT=lambda k, t=in2: t[:, 2 * k + 1],
                    UT=lambda k, t=in2: t[:, 2 * KB + k],
                    AM8=lambda k, t=in8: t[:, k],
                    A8pair=lambda j, t=in8: t[:, KB + 2 * j:KB + 2 * j + 2],
                    L8pair=lambda j, r, t=in8: t[:, 2 * KB + 2 * j:
                                                 2 * KB + 2 * j + 2,
                                                 r * 128:(r + 1) * 128],
                )

            def sec_m1(b, V):
                """M1: UTG = u^T G (k-outer) ; W = 0.5*amat - UTG (DVE)."""
                if "UT8pair" in V:
                    # batch 1: W is written in fp8 so u@W can run DoubleRow
                    # (W's k-blocks are pair-adjacent in this layout)
                    w_sb = midp.tile([128, KB, P], F8, tag="w8", bufs=1,
                                     name=f"w8_{b}")
                else:
                    w_sb = midp.tile([128, KB, P], BF16, tag="w",
                                     name=f"w_{b}")
                utg = [psum.tile([128, P], F32, tag="ps", name=f"utg{b}_{r}")
                       for r in range(CH)]
                if "U8pair" in V:
                    for j in range(2):
                        for r in range(CH):
                            nc.tensor.matmul(utg[r][:], V["U8pair"](j, r),
                                             V["G8pair"](j), perf_mode=DR,
                                             start=(j == 0), stop=(j == 1))
                elif "U8H" in V:
                    # batch 0 hybrid: k=0,1 as plain fp8 passes (each gated on
                    # a 128KB head piece, so compute starts at first-piece
                    # arrival), then the (u2,u3) k-pair as one DR pass
                    for k in range(2):
                        for r in range(CH):
                            nc.tensor.matmul(utg[r][:],
                                             V["Un"](k)[:, r * 128:(r + 1) * 128],
                                             V["Gn"](k)[:], start=(k == 0),
                                             stop=False)
                    for r in range(CH):
                        nc.tensor.matmul(utg[r][:], V["U8H"](r), V["G8H"](),
                                         perf_mode=DR, start=False, stop=True,
                                         skip_group_check=True)
                else:
                    for k in range(KB):
                        for r in range(CH):
                            nc.tensor.matmul(utg[r][:],
                                             V["Un"](k)[:, r * 128:(r + 1) * 128],
                                             V["Gn"](k)[:], start=(k == 0),
                                             stop=(k == KB - 1))
                # batch 1: u8 is shipped pre-scaled by 1/4 (exact exponent
                # shift), so utg = UTG/4 and W8 = W/4 stays inside fp8e4's
                # +-240 range (|W| itself reaches ~590); the du add scales
                # the matmul result back by 4.
                wscale = 0.125 if "UT8pair" in V else 0.5
                for r in range(CH):
                    nc.vector.scalar_tensor_tensor(w_sb[:, r], V["AM8"](r)[:],
                                                   wscale, utg[r][:], AOP.mult,
                                                   AOP.subtract)
                return w_sb

            def sec_m5(b, V):
                """M5: S = lam @ G^T (k-outer)."""
                s_sb = midp.tile([128, KB, N], BF16, tag="s", name=f"s_{b}")
                s_ps = [psum.tile([128, N], F32, tag="ps", name=f"s{b}_{r}")
                        for r in range(CH)]
                if "GT8pair" in V:
                    for j in range(2):
                        for r in range(CH):
                            nc.tensor.matmul(s_ps[r][:], V["L8pair"](j, r),
                                             V["GT8pair"](j), perf_mode=DR,
                                             start=(j == 0), stop=(j == 1))
                else:
                    for k in range(KB):
                        for r in range(CH):
                            nc.tensor.matmul(s_ps[r][:],
                                             V["LT"](k)[:, r * 128:(r + 1) * 128],
                                             V["GT"](k)[:], start=(k == 0),
                                             stop=(k == KB - 1))
                for r in range(CH):
                    nc.scalar.copy(s_sb[:, r], s_ps[r][:])
                return s_sb

            def sec_m23(b, V, w_sb):
                """M23: du = u @ W + G, stored out."""
                du_sb = outsp.tile([128, KB, P], BF16, tag="du", name=f"du_{b}")
                for r in range(CH):
                    ps = psum.tile([128, P], F32, tag="ps", name=f"du{b}_{r}")
                    if "UT8pair" in V:
                        for j in range(2):
                            nc.tensor.matmul(ps[:], V["UT8pair"](j, r),
                                             w_sb[:, 2 * j:2 * j + 2],
                                             perf_mode=DR, start=(j == 0),
                                             stop=(j == 1))
                    else:
                        for k in range(KB):
                            nc.tensor.matmul(ps[:],
                                             V["UT"](k)[:, r * 128:(r + 1) * 128],
                                             w_sb[:, k], start=(k == 0),
                                             stop=(k == KB - 1))
                    if "UT8pair" in V:
                        nc.vector.scalar_tensor_tensor(du_sb[:, r], ps[:],
                                                       4.0, V["Gn"](r)[:],
                                                       AOP.mult, AOP.add)
                    else:
                        nc.vector.tensor_tensor(du_sb[:, r], ps[:],
                                                V["Gn"](r)[:], AOP.add)
                    if b == BLOC - 1:
                        nc.scalar.dma_start(d_du[b][:, r], du_sb[:, r])
                if b < BLOC - 1:
                    nc.scalar.dma_start(d_du[b], du_sb[:])

            def sec_tail(b, V, s_sb):
                """C = S + S^T, then dlam = lam @ A (fp8 DR) + C @ u."""
                coup_sb = midp.tile([128, KB, N], BF16, tag="coup",
                                    name=f"coup_{b}")
                for r in range(CH):
                    tps = psum.tile([128, N], BF16, tag="tps", bufs=2,
                                    name=f"tps{b}_{r}")
                    for c in range(KB):
                        nc.tensor.transpose(tps[:, c * 128:(c + 1) * 128],
                                            s_sb[:, c, r * 128:(r + 1) * 128],
                                            identb[:])
                    nc.vector.tensor_tensor(coup_sb[:, r], tps[:], s_sb[:, r],
                                            AOP.add)

                dlam_sb = outsp.tile([128, KB, P], BF16, tag="dlam",
                                     name=f"dlam_{b}")
                dlam_ps = [psum.tile([128, P], F32, tag="ps", name=f"dl{b}_{r}")
                           for r in range(CH)]
                for r in range(CH):
                    for j in range(2):
                        nc.tensor.matmul(dlam_ps[r][:], V["L8pair"](j, r),
                                         V["A8pair"](j), perf_mode=DR,
                                         start=(j == 0), stop=False,
                                         skip_group_check=True)
                for r in range(CH):
                    ps = dlam_ps[r]
                    for k in range(KB):
                        nc.tensor.matmul(ps[:],
                                         coup_sb[:, k, r * 128:(r + 1) * 128],
                                         V["Un"](k)[:], start=False,
                                         stop=(k == KB - 1),
                                         skip_group_check=True)
                    # alternate copy engines so the copies drain in parallel;
                    # the LAST chunk's copy goes to the (faster) vector engine
                    # so the final store issues as early as possible
                    if r % 2 == 0:
                        nc.scalar.copy(dlam_sb[:, r], ps[:])
                    else:
                        nc.vector.tensor_copy(dlam_sb[:, r], ps[:])
                    if b == BLOC - 1:
                        if r < CH - 1:
                            qq = nc.sync if r % 2 == 0 else nc.scalar
                            qq.dma_start(d_dlam[b][:, r], dlam_sb[:, r])
                        else:
                            # split the very last store across both HWDGE
                            # rings so its halves drain in parallel and the
                            # end-of-kernel wait sees a 64KB receipt, not 128KB
                            nc.sync.dma_start(d_dlam[b][:, r, 0:256],
                                              dlam_sb[:, r, 0:256])
                            nc.scalar.dma_start(d_dlam[b][:, r, 256:512],
                                                dlam_sb[:, r, 256:512])
                if b < BLOC - 1:
                    nc.scalar.dma_start(d_dlam[b], dlam_sb[:])

            # all batches run SOLO: within one batch the section chain
            # M1 -> M5 -> M23 -> tail already overlaps every cross-engine
            # handoff (W's DVE pass runs under M5, the S copies under M23,
            # the coupling add under the tail's DR matmuls), and solo
            # sequencing needs each batch's inputs ~5us later than pairing —
            # decisive, because the head DMA stream measures fully saturated
            # (~357 GB/s) and the PE otherwise catches up with it around b3.
            for group in [(b,) for b in range(BLOC)]:
                Vs = [mk_views(b) for b in group]
                ws = [sec_m1(b, V) for b, V in zip(group, Vs)]
                ss = [sec_m5(b, V) for b, V in zip(group, Vs)]
                for b, V, w in zip(group, Vs, ws):
                    sec_m23(b, V, w)
                for b, V, s in zip(group, Vs, ss):
                    sec_tail(b, V, s)

    nc.compile()
    return nc


_NC = None


def _pack(x, dt):
    """[BLOC,512,512] -> [BLOC,128,4,512] in SBUF layout (partition-major)."""
    return np.ascontiguousarray(
        x.reshape(BLOC, KB, 128, P).transpose(0, 2, 1, 3).astype(dt))


def _unpack(y):
    """[BLOC,128,4,512] bf16 -> [BLOC,512,512] fp32."""
    return y.transpose(0, 2, 1, 3).reshape(BLOC, N, P).astype(np.float32)


def _make_in_maps(u, lam, A, G):
    u = np.asarray(u, dtype=np.float32)
    lam = np.asarray(lam, dtype=np.float32)
    A = np.asarray(A, dtype=np.float32)
    G = np.asarray(G, dtype=np.float32)

    in_maps = []
    for c in range(NCORES):
        sl = slice(c * BLOC, (c + 1) * BLOC)
        uc, lamc, Ac, Gc = u[sl], lam[sl], A[sl], G[sl]
        At = np.swapaxes(Ac, 1, 2)
        lamt = np.swapaxes(lamc, 1, 2)
        in1 = np.empty((BLOC, 128, 2 * KB, P), dtype=NP_BF16)
        in1[:, :, 0::2] = _pack(uc, NP_BF16)
        in1[:, :, 1::2] = _pack(Gc, NP_BF16)
        in2 = np.empty((BLOC, 128, 3 * KB, P), dtype=NP_BF16)
        in2[:, :, 0:2 * KB:2] = _pack(lamt, NP_BF16)
        in2[:, :, 1:2 * KB:2] = _pack(np.swapaxes(Gc, 1, 2), NP_BF16)
        in2[:, :, 2 * KB:] = _pack(np.swapaxes(uc, 1, 2), NP_BF16)
        # lamt8 is quantized from the bf16 lamt (same value chain as on-device)
        in8 = np.concatenate([_pack(Ac - At, NP_F8), _pack(Ac, NP_F8),
                              _pack(lamt, NP_BF16).astype(NP_F8)], axis=2)
        # batch-1 fp8 DoubleRow operands, quantized from the same bf16 chain:
        # [u8/4 | g8 | gt8 | ut8] — u8 quantized then scaled by 1/4 (an
        # exact exponent shift) so the on-device W/4 fits fp8e4's range
        u8q = in1[1][:, 0::2].astype(NP_F8).astype(np.float32) / 4.0
        x8b1 = np.ascontiguousarray(np.concatenate(
            [u8q, in1[1][:, 1::2].astype(np.float32),
             in2[1][:, 1:2 * KB:2].astype(np.float32),
             in2[1][:, 2 * KB:].astype(np.float32)], axis=1)).astype(NP_F8)
        # h1f8: [u0,g0 | u1,g1 | u2,u3,g2,g3]; h2f8: [gt0..3 | ut0..3]
        h1f8 = np.ascontiguousarray(
            in1[0][:, [0, 1, 2, 3, 4, 6, 5, 7]]).astype(NP_F8)
        h2f8 = np.ascontiguousarray(
            in2[0][:, [1, 3, 5, 7, 8, 9, 10, 11]]).astype(NP_F8)
        in_maps.append({"in1": in1, "in2": in2, "in8": in8, "x8b1": x8b1,
                        "h2f8": h2f8, "h1f8": h1f8})
    return in_maps


def kernel(u, lam, A, G, t=None, **_ignored):
    global _NC
    if _NC is None:
        _NC = _build_nc()
    nc = _NC

    in_maps = _make_in_maps(u, lam, A, G)
    res = run_bass_kernel_spmd(nc, in_maps, list(range(NCORES)))
    du = np.concatenate([_unpack(res.results[c]["du"]) for c in range(NCORES)],
                        axis=0)
    dlam = np.concatenate([_unpack(res.results[c]["dlam"])
                           for c in range(NCORES)], axis=0)
    return du, dlam



# revision 45
# speedup vs baseline: 1.0063x; 1.0005x over previous
"""Trainium2 Bass kernel for nn_AugmentedODE (B=64, N=P=512), 8-core data parallel.

Per batch the reference computes (7 matmuls of 512^3):
    Omega   = 0.5*(A - A^T)
    du      = u @ Omega + G - u @ (u^T G)
    S       = lam @ G^T
    dlam    = lam @ A + (S + S^T) @ u

Restructured to 5 matmuls + 1 PE transpose set per batch:
    UTG = u^T G                      (bf16:  lhsT=u,    rhs=G)
    W   = 0.5*(A - A^T) - UTG        (DVE; A - A^T precomputed host-side, fp8)
    du  = u @ W + G                  (bf16:  lhsT=u^T,  rhs=W; +G fused in DVE)
    S   = lam @ G^T                  (bf16:  lhsT=lam^T, rhs=G^T)
    C   = S + S^T                    (bf16 PE transpose + DVE add)
    dlam= lam @ A + C @ u            (lam@A in fp8 DoubleRow at 2x rate; C@u bf16;
                                      both accumulated into one PSUM group)

Rel-err budget is 2e-2 (Frobenius); measured 1.734e-2 for this mix (du
1.734e-2, dlam 1.689e-2 — deterministic, seed-0 inputs).  The four
magnitude-dominant matmuls (UTG, u@W, S, C@u) stay bf16 for batches 2-7 and
run at the PE's bf16 roofline (1 row/cycle, ~216 ns per 128x128x512 matmul).
Batch 0 is fully fp8 (halves its head DMA) and runs its S matmul + half its
UTG in fp8 DoubleRow (numerically identical to its plain-fp8 form, ~2.4us
PE saved).  Batch 1 runs UTG, S AND u@W in fp8 DoubleRow (~4.8us PE saved);
u8 is shipped pre-scaled by 1/4 (exact) so the on-device W/4 fits fp8e4's
+-240 range (|W| itself reaches ~590 — product-of-Gaussians tails), and the
du add rescales by 4.  lam@A (~3% of |dlam|) and the skew term (~5% of |W|)
are fp8 DR everywhere.  The error budget is now ~87% consumed; converting
any further bf16 matmul instance to fp8 (+~1e-4 variance each) would cross
the 2e-2 gate.

(Note: offloading the S+S^T transposes to the DMA xbar was tried and
reverted — Tile globally serializes dma-transposes against ALL in-flight
HWDGE DMAs as a HW-deadlock guard, which strings them out by 15-25us and
starves the PE.  The 16 PE transposes per batch stream at ~56ns each and
are the cheapest correct option.)

All operands are pre-packed on the host into the exact SBUF layout
([128 partitions, kblock, 512] with k-blocks contiguous per partition) and
concatenated into three blobs per batch, so every DMA line is multi-KB
contiguous on both sides.  Dependency tracking is per-tile, so batch 0 uses
peeled per-piece tiles whose DMAs are sequenced on one queue in consumption
order; batches 1-3 stream whole blobs on the same queue behind them; later
batches prefetch on parallel queues, gated naturally by the 4-deep input
rings.  Batches are processed in PAIRS with sections interleaved
(M1(b), M1(b+1), M5(b), M5(b+1), ...) so every section boundary is followed
by independent work and cross-engine handoff latencies stay off the PE
critical path.  Outputs are written bf16 and upcast on the host.
"""
import numpy as np
import ml_dtypes

import concourse.bass as bass
import concourse.mybir as mybir
import concourse.tile as tile
from concourse import bacc
from concourse.bass_utils import run_bass_kernel_spmd
from concourse.masks import make_identity

F32 = mybir.dt.float32
F32R = mybir.dt.float32r
BF16 = mybir.dt.bfloat16
F8 = mybir.dt.float8e4
AOP = mybir.AluOpType
DR = mybir.MatmulPerfMode.DoubleRow

NP_BF16 = ml_dtypes.bfloat16
NP_F8 = ml_dtypes.float8_e4m3

B, N, P = 64, 512, 512
NCORES = 8
BLOC = B // NCORES          # batches per core
KB = 4                      # 512 = 4 k-blocks of 128
CH = 4                      # 4 output chunks of 128 rows
WARMUP_MM = 5


def _build_nc():
    nc = bacc.Bacc("TRN2", target_bir_lowering=False, debug=False,
                   num_devices=NCORES)

    # in1: interleaved [u0,g0,u1,g1,u2,g2,u3,g3]
    # in2: interleaved [lamt0,gt0,...,lamt3,gt3, ut0..ut3]
    # in8: amat=A-A^T(0:4) | a(4:8) | lamt8(8:12), fp8e4
    d_in1 = nc.declare_dram_parameter("in1", [BLOC, 128, 2 * KB, P], BF16,
                                      isOutput=False)
    d_in2 = nc.declare_dram_parameter("in2", [BLOC, 128, 3 * KB, P], BF16,
                                      isOutput=False)
    d_in8 = nc.declare_dram_parameter("in8", [BLOC, 128, 3 * KB, P], F8,
                                      isOutput=False)
    # batch-0 fp8 head blobs: h2f8 = [gt0..3 | ut0..3] (its lamt fp8 comes
    # from in8's lamt8 blocks, so it is not shipped twice), h1f8 =
    # [u0,g0 | u1,g1 | u2,u3,g2,g3] (last four blocks reordered so the
    # (u2,u3)x(g2,g3) k-pair runs as one fp8 DoubleRow pass)
    d_h2f8 = nc.declare_dram_parameter("h2f8", [128, 2 * KB, P], F8,
                                       isOutput=False)
    d_h1f8 = nc.declare_dram_parameter("h1f8", [128, 2 * KB, P], F8,
                                       isOutput=False)
    # batch-1 fp8 operands for its DoubleRow UTG, S and u@W matmuls:
    # [u8(0:4) | g8(4:8) | gt8(8:12) | ut8(12:16)]
    d_x8b1 = nc.declare_dram_parameter("x8b1", [128, 4 * KB, P], F8,
                                       isOutput=False)
    d_du = nc.declare_dram_parameter("du", [BLOC, 128, KB, P], BF16,
                                     isOutput=True)
    d_dlam = nc.declare_dram_parameter("dlam", [BLOC, 128, KB, P], BF16,
                                       isOutput=True)

    with tile.TileContext(nc) as tc:
        with (
            tc.tile_pool(name="const", bufs=1) as constp,
            tc.tile_pool(name="ins", bufs=4) as insp,
            tc.tile_pool(name="mid", bufs=2) as midp,
            tc.tile_pool(name="outs", bufs=2) as outsp,
            tc.tile_pool(name="psum", bufs=6, space="PSUM") as psum,
        ):
            # warm-up source first: one memset, no identity dependency, so the
            # PE's ~6us low-clock window starts at the earliest possible point
            # memset on DVE: gpsimd is busy with its framework preamble
            # (DMA rings, library loads) until ~7us, which would delay the
            # first warm-up matmul and with it the HAM un-throttle point
            wsrc = constp.tile([128, 512], BF16)
            nc.vector.memset(wsrc[:].bitcast(F32), 0.0)
            warm_ps = psum.tile([128, 512], F32, tag="ps")
            for i in range(WARMUP_MM):
                nc.tensor.matmul(warm_ps[:], wsrc[:, 0:128], wsrc[:],
                                 start=True, stop=True)

            identf = constp.tile([128, 128], F32)
            make_identity(nc, identf[:])
            identb = constp.tile([128, 128], BF16)
            nc.gpsimd.tensor_copy(identb[:], identf[:])

            # batch-0 peeled tiles: per-piece DMAs on ONE queue in consumption
            # order (dep tracking is per-tile, so fine pieces start compute early)
            h1a = insp.tile([128, 2, P], F8, tag="h1a", bufs=1)        # u0,g0
            h1b1 = insp.tile([128, 2, P], F8, tag="h1b1", bufs=1)      # u1,g1
            h1b2 = insp.tile([128, 4, P], F8, tag="h1b2", bufs=1)      # u2,u3,g2,g3
            h8a = insp.tile([128, KB, P], F8, tag="h8a", bufs=1)       # amat
            hgt = insp.tile([128, KB, P], F8, tag="hgt", bufs=1)       # gt0..3
            h8b2 = insp.tile([128, KB, P], F8, tag="h8b2", bufs=1)     # lamt8
            h2c = insp.tile([128, KB, P], F8, tag="h2c", bufs=1)       # ut
            h8b1 = insp.tile([128, KB, P], F8, tag="h8b1", bufs=1)     # a
            # consumption order: M1's u/g pieces, M5-DR's gt8+lamt8, only
            # THEN the W skew term (the DVE STT runs after M1's PSUM drains
            # anyway), M23's ut8, and the tail's a-blocks last.  Putting
            # amat before gt8/lamt8 was measured as a 0.8-2.3us PE stall:
            # M5(b0) sat waiting while bytes it didn't need yet streamed.
            nc.sync.dma_start(h1a[:], d_h1f8[:, 0:2])
            nc.sync.dma_start(h1b1[:], d_h1f8[:, 2:4])
            nc.sync.dma_start(h1b2[:], d_h1f8[:, 4:2 * KB])
            nc.sync.dma_start(hgt[:], d_h2f8[:, 0:KB])
            nc.sync.dma_start(h8b2[:], d_in8[0][:, 2 * KB:3 * KB])
            nc.sync.dma_start(h8a[:], d_in8[0][:, 0:KB])
            nc.sync.dma_start(h2c[:], d_h2f8[:, KB:2 * KB])
            nc.sync.dma_start(h8b1[:], d_in8[0][:, KB:2 * KB])

            def mk_views(b):
                """Allocate input tiles (b>0), emit their DMAs, return accessors."""
                if b == 0:
                    h1 = [h1a, h1b1, h1b2, h1b2]
                    h1o = [0, 0, 0, 1]
                    g1o = [1, 1, 2, 3]
                    return dict(
                        Un=lambda k: h1[k][:, h1o[k]],
                        Gn=lambda k: h1[k][:, g1o[k]],
                        UT=lambda k: h2c[:, k],
                        AM8=lambda k: h8a[:, k],
                        A8pair=lambda j: h8b1[:, 2 * j:2 * j + 2],
                        L8pair=lambda j, r: h8b2[:, 2 * j:2 * j + 2,
                                                 r * 128:(r + 1) * 128],
                        GT8pair=lambda j: hgt[:, 2 * j:2 * j + 2],
                        U8H=lambda r: h1b2[:, 0:2, r * 128:(r + 1) * 128],
                        G8H=lambda: h1b2[:, 2:4],
                    )
                if b == 1:
                    # batch 1 computes UTG, S, AND u@W in fp8 DoubleRow from
                    # x8b1 (= [u8 | g8 | gt8 | ut8]); its bf16 lamt/gt/ut are
                    # never read, nor is its bf16 G (the du "+G" add reads
                    # g8 — G is ~1/512 of |du|, so fp8 G costs nothing).
                    # Only the u half of in1[1] is still loaded in bf16 (the
                    # C@u right operand, which feeds the dominant dlam term).
                    # (Peeling these blobs into finer per-section pieces was
                    # tried and reverted: the head stream is bandwidth-bound,
                    # so pieces only redistribute the same wait while adding
                    # per-piece completion-latency jitter.)
                    in8 = insp.tile([128, 3 * KB, P], F8, tag="in8",
                                    name="in8_1")
                    x8 = insp.tile([128, 4 * KB, P], F8, tag="x8b1", bufs=1)
                    in1u = insp.tile([128, KB, P], BF16, tag="in1b1", bufs=1)
                    nc.sync.dma_start(x8[:], d_x8b1[:])
                    nc.sync.dma_start(in8[:], d_in8[1])
                    nc.sync.dma_start(in1u[:], d_in1[1][:, 0:2 * KB:2])
                    return dict(
                        Un=lambda k, t=in1u: t[:, k],
                        Gn=lambda k, t=x8: t[:, KB + k],
                        AM8=lambda k, t=in8: t[:, k],
                        A8pair=lambda j, t=in8: t[:, KB + 2 * j:KB + 2 * j + 2],
                        L8pair=lambda j, r, t=in8: t[:, 2 * KB + 2 * j:
                                                     2 * KB + 2 * j + 2,
                                                     r * 128:(r + 1) * 128],
                        U8pair=lambda j, r, t=x8: t[:, 2 * j:2 * j + 2,
                                                    r * 128:(r + 1) * 128],
                        G8pair=lambda j, t=x8: t[:, KB + 2 * j:KB + 2 * j + 2],
                        GT8pair=lambda j, t=x8: t[:, 2 * KB + 2 * j:
                                                  2 * KB + 2 * j + 2],
                        UT8pair=lambda j, r, t=x8: t[:, 3 * KB + 2 * j:
                                                     3 * KB + 2 * j + 2,
                                                     r * 128:(r + 1) * 128],
                    )
                # bufs=3: only b2..b7 allocate from these rings (b1 uses
                # private slim tiles), so 3 slots restore the "b5 gated on
                # b2's release" back-pressure that keeps late batches' loads
                # from stealing head DMA bandwidth.
                in1 = insp.tile([128, 2 * KB, P], BF16, tag="in1", bufs=3,
                                name=f"in1_{b}")
                in2 = insp.tile([128, 3 * KB, P], BF16, tag="in2", bufs=3,
                                name=f"in2_{b}")
                in8 = insp.tile([128, 3 * KB, P], F8, tag="in8",
                                name=f"in8_{b}")
                if b <= 4:
                    # head batches: ordered on sync behind batch 0's pieces,
                    # so nothing dilutes the stream batch 0/1 are waiting on
                    nc.sync.dma_start(in1[:], d_in1[b])
                    nc.sync.dma_start(in8[:], d_in8[b])
                    nc.sync.dma_start(in2[:], d_in2[b])
                else:
                    # b>=5 are ring-gated behind live tiles (slot frees only
                    # after batch b-4 completes), so parallel queues can't
                    # steal head bandwidth
                    nc.sync.dma_start(in1[:], d_in1[b])
                    nc.gpsimd.dma_start(in2[:], d_in2[b])
                    nc.gpsimd.dma_start(in8[:], d_in8[b])
                return dict(
                    Un=lambda k, t=in1: t[:, 2 * k],
                    Gn=lambda k, t=in1: t[:, 2 * k + 1],
                    LT=lambda k, t=in2: t[:, 2 * k],
                    GT=lambda k, t=in2: t[:, 2 * k + 1],
                    UT=lambda k, t=in2: t[:, 2 * KB + k],
                    AM8=lambda k, t=in8: t[:, k],
                    A8pair=lambda j, t=in8: t[:, KB + 2 * j:KB + 2 * j + 2],
                    L8pair=lambda j, r, t=in8: t[:, 2 * KB + 2 * j:
                                                 2 * KB + 2 * j + 2,
                                                 r * 128:(r + 1) * 128],
                )

            def sec_m1(b, V):
                """M1: UTG = u^T G (k-outer) ; W = 0.5*amat - UTG (DVE)."""
                if "UT8pair" in V:
                    # batch 1: W is written in fp8 so u@W can run DoubleRow
                    # (W's k-blocks are pair-adjacent in this layout)
                    w_sb = midp.tile([128, KB, P], F8, tag="w8", bufs=1,
                                     name=f"w8_{b}")
                else:
                    w_sb = midp.tile([128, KB, P], BF16, tag="w",
                                     name=f"w_{b}")
                utg = [psum.tile([128, P], F32, tag="ps", name=f"utg{b}_{r}")
                       for r in range(CH)]
                if "U8pair" in V:
                    for j in range(2):
                        for r in range(CH):
                            nc.tensor.matmul(utg[r][:], V["U8pair"](j, r),
                                             V["G8pair"](j), perf_mode=DR,
                                             start=(j == 0), stop=(j == 1))
                elif "U8H" in V:
                    # batch 0 hybrid: k=0,1 as plain fp8 passes (each gated on
                    # a 128KB head piece, so compute starts at first-piece
                    # arrival), then the (u2,u3) k-pair as one DR pass
                    for k in range(2):
                        for r in range(CH):
                            nc.tensor.matmul(utg[r][:],
                                             V["Un"](k)[:, r * 128:(r + 1) * 128],
                                             V["Gn"](k)[:], start=(k == 0),
                                             stop=False)
                    for r in range(CH):
                        nc.tensor.matmul(utg[r][:], V["U8H"](r), V["G8H"](),
                                         perf_mode=DR, start=False, stop=True,
                                         skip_group_check=True)
                else:
                    for k in range(KB):
                        for r in range(CH):
                            nc.tensor.matmul(utg[r][:],
                                             V["Un"](k)[:, r * 128:(r + 1) * 128],
                                             V["Gn"](k)[:], start=(k == 0),
                                             stop=(k == KB - 1))
                # batch 1: u8 is shipped pre-scaled by 1/4 (exact exponent
                # shift), so utg = UTG/4 and W8 = W/4 stays inside fp8e4's
                # +-240 range (|W| itself reaches ~590); the du add scales
                # the matmul result back by 4.
                wscale = 0.125 if "UT8pair" in V else 0.5
                for r in range(CH):
                    nc.vector.scalar_tensor_tensor(w_sb[:, r], V["AM8"](r)[:],
                                                   wscale, utg[r][:], AOP.mult,
                                                   AOP.subtract)
                return w_sb

            def sec_m5(b, V):
                """M5: S = lam @ G^T (k-outer)."""
                s_sb = midp.tile([128, KB, N], BF16, tag="s", name=f"s_{b}")
                s_ps = [psum.tile([128, N], F32, tag="ps", name=f"s{b}_{r}")
                        for r in range(CH)]
                if "GT8pair" in V:
                    for j in range(2):
                        for r in range(CH):
                            nc.tensor.matmul(s_ps[r][:], V["L8pair"](j, r),
                                             V["GT8pair"](j), perf_mode=DR,
                                             start=(j == 0), stop=(j == 1))
                else:
                    for k in range(KB):
                        for r in range(CH):
                            nc.tensor.matmul(s_ps[r][:],
                                             V["LT"](k)[:, r * 128:(r + 1) * 128],
                                             V["GT"](k)[:], start=(k == 0),
                                             stop=(k == KB - 1))
                for r in range(CH):
                    nc.scalar.copy(s_sb[:, r], s_ps[r][:])
                return s_sb

            def sec_m23(b, V, w_sb):
                """M23: du = u @ W + G, stored out."""
                du_sb = outsp.tile([128, KB, P], BF16, tag="du", name=f"du_{b}")
                for r in range(CH):
                    ps = psum.tile([128, P], F32, tag="ps", name=f"du{b}_{r}")
                    if "UT8pair" in V:
                        for j in range(2):
                            nc.tensor.matmul(ps[:], V["UT8pair"](j, r),
                                             w_sb[:, 2 * j:2 * j + 2],
                                             perf_mode=DR, start=(j == 0),
                                             stop=(j == 1))
                    else:
                        for k in range(KB):
                            nc.tensor.matmul(ps[:],
                                             V["UT"](k)[:, r * 128:(r + 1) * 128],
                                             w_sb[:, k], start=(k == 0),
                                             stop=(k == KB - 1))
                    if "UT8pair" in V:
                        nc.vector.scalar_tensor_tensor(du_sb[:, r], ps[:],
                                                       4.0, V["Gn"](r)[:],
                                                       AOP.mult, AOP.add)
                    else:
                        nc.vector.tensor_tensor(du_sb[:, r], ps[:],
                                                V["Gn"](r)[:], AOP.add)
                    if b == BLOC - 1:
                        nc.scalar.dma_start(d_du[b][:, r], du_sb[:, r])
                if b < BLOC - 1:
                    nc.scalar.dma_start(d_du[b], du_sb[:])

            def sec_tail(b, V, s_sb):
                """C = S + S^T, then dlam = lam @ A (fp8 DR) + C @ u."""
                coup_sb = midp.tile([128, KB, N], BF16, tag="coup",
                                    name=f"coup_{b}")
                for r in range(CH):
                    tps = psum.tile([128, N], BF16, tag="tps", bufs=2,
                                    name=f"tps{b}_{r}")
                    for c in range(KB):
                        nc.tensor.transpose(tps[:, c * 128:(c + 1) * 128],
                                            s_sb[:, c, r * 128:(r + 1) * 128],
                                            identb[:])
                    nc.vector.tensor_tensor(coup_sb[:, r], tps[:], s_sb[:, r],
                                            AOP.add)

                dlam_sb = outsp.tile([128, KB, P], BF16, tag="dlam",
                                     name=f"dlam_{b}")
                dlam_ps = [psum.tile([128, P], F32, tag="ps", name=f"dl{b}_{r}")
                           for r in range(CH)]
                for r in range(CH):
                    for j in range(2):
                        nc.tensor.matmul(dlam_ps[r][:], V["L8pair"](j, r),
                                         V["A8pair"](j), perf_mode=DR,
                                         start=(j == 0), stop=False,
                                         skip_group_check=True)
                for r in range(CH):
                    ps = dlam_ps[r]
                    for k in range(KB):
                        nc.tensor.matmul(ps[:],
                                         coup_sb[:, k, r * 128:(r + 1) * 128],
                                         V["Un"](k)[:], start=False,
                                         stop=(k == KB - 1),
                                         skip_group_check=True)
                    # alternate copy engines so the copies drain in parallel;
                    # the very LAST chunk's copy is split across both copy
                    # engines so the final stores issue as early as possible
                    if b == BLOC - 1 and r == CH - 1:
                        nc.vector.tensor_copy(dlam_sb[:, r, 0:256],
                                              ps[:, 0:256])
                        nc.scalar.copy(dlam_sb[:, r, 256:512], ps[:, 256:512])
                    elif r % 2 == 0:
                        nc.scalar.copy(dlam_sb[:, r], ps[:])
                    else:
                        nc.vector.tensor_copy(dlam_sb[:, r], ps[:])
                    if b == BLOC - 1:
                        if r < CH - 1:
                            qq = nc.sync if r % 2 == 0 else nc.scalar
                            qq.dma_start(d_dlam[b][:, r], dlam_sb[:, r])
                        else:
                            # split the very last store across both HWDGE
                            # rings so its halves drain in parallel and the
                            # end-of-kernel wait sees a 64KB receipt, not 128KB
                            nc.sync.dma_start(d_dlam[b][:, r, 0:256],
                                              dlam_sb[:, r, 0:256])
                            nc.scalar.dma_start(d_dlam[b][:, r, 256:512],
                                                dlam_sb[:, r, 256:512])
                if b < BLOC - 1:
                    nc.scalar.dma_start(d_dlam[b], dlam_sb[:])

            # all batches run SOLO: within one batch the section chain
            # M1 -> M5 -> M23 -> tail already overlaps every cross-engine
            # handoff (W's DVE pass runs under M5, the S copies under M23,
            # the coupling add under the tail's DR matmuls), and solo
            # sequencing needs each batch's inputs ~5us later than pairing —
            # decisive, because the head DMA stream measures fully saturated
            # (~357 GB/s) and the PE otherwise catches up with it around b3.
            for group in [(b,) for b in range(BLOC)]:
                Vs = [mk_views(b) for b in group]
                ws = [sec_m1(b, V) for b, V in zip(group, Vs)]
                ss = [sec_m5(b, V) for b, V in zip(group, Vs)]
                for b, V, w in zip(group, Vs, ws):
                    sec_m23(b, V, w)
                for b, V, s in zip(group, Vs, ss):
                    sec_tail(b, V, s)

    nc.compile()
    return nc


_NC = None


def _pack(x, dt):
    """[BLOC,512,512] -> [BLOC,128,4,512] in SBUF layout (partition-major)."""
    return np.ascontiguousarray(
        x.reshape(BLOC, KB, 128, P).transpose(0, 2, 1, 3).astype(dt))


def _unpack(y):
    """[BLOC,128,4,512] bf16 -> [BLOC,512,512] fp32."""
    return y.transpose(0, 2, 1, 3).reshape(BLOC, N, P).astype(np.float32)


def _make_in_maps(u, lam, A, G):
    u = np.asarray(u, dtype=np.float32)
    lam = np.asarray(lam, dtype=np.float32)
    A = np.asarray(A, dtype=np.float32)
    G = np.asarray(G, dtype=np.float32)

    in_maps = []
    for c in range(NCORES):
        sl = slice(c * BLOC, (c + 1) * BLOC)
        uc, lamc, Ac, Gc = u[sl], lam[sl], A[sl], G[sl]
        At = np.swapaxes(Ac, 1, 2)
        lamt = np.swapaxes(lamc, 1, 2)
        in1 = np.empty((BLOC, 128, 2 * KB, P), dtype=NP_BF16)
        in1[:, :, 0::2] = _pack(uc, NP_BF16)
        in1[:, :, 1::2] = _pack(Gc, NP_BF16)
        in2 = np.empty((BLOC, 128, 3 * KB, P), dtype=NP_BF16)
        in2[:, :, 0:2 * KB:2] = _pack(lamt, NP_BF16)
        in2[:, :, 1:2 * KB:2] = _pack(np.swapaxes(Gc, 1, 2), NP_BF16)
        in2[:, :, 2 * KB:] = _pack(np.swapaxes(uc, 1, 2), NP_BF16)
        # lamt8 is quantized from the bf16 lamt (same value chain as on-device)
        in8 = np.concatenate([_pack(Ac - At, NP_F8), _pack(Ac, NP_F8),
                              _pack(lamt, NP_BF16).astype(NP_F8)], axis=2)
        # batch-1 fp8 DoubleRow operands, quantized from the same bf16 chain:
        # [u8/4 | g8 | gt8 | ut8] — u8 quantized then scaled by 1/4 (an
        # exact exponent shift) so the on-device W/4 fits fp8e4's range
        u8q = in1[1][:, 0::2].astype(NP_F8).astype(np.float32) / 4.0
        x8b1 = np.ascontiguousarray(np.concatenate(
            [u8q, in1[1][:, 1::2].astype(np.float32),
             in2[1][:, 1:2 * KB:2].astype(np.float32),
             in2[1][:, 2 * KB:].astype(np.float32)], axis=1)).astype(NP_F8)
        # h1f8: [u0,g0 | u1,g1 | u2,u3,g2,g3]; h2f8: [gt0..3 | ut0..3]
        h1f8 = np.ascontiguousarray(
            in1[0][:, [0, 1, 2, 3, 4, 6, 5, 7]]).astype(NP_F8)
        h2f8 = np.ascontiguousarray(
            in2[0][:, [1, 3, 5, 7, 8, 9, 10, 11]]).astype(NP_F8)
        in_maps.append({"in1": in1, "in2": in2, "in8": in8, "x8b1": x8b1,
                        "h2f8": h2f8, "h1f8": h1f8})
    return in_maps


def kernel(u, lam, A, G, t=None, **_ignored):
    global _NC
    if _NC is None:
        _NC = _build_nc()
    nc = _NC

    in_maps = _make_in_maps(u, lam, A, G)
    res = run_bass_kernel_spmd(nc, in_maps, list(range(NCORES)))
    du = np.concatenate([_unpack(res.results[c]["du"]) for c in range(NCORES)],
                        axis=0)
    dlam = np.concatenate([_unpack(res.results[c]["dlam"])
                           for c in range(NCORES)], axis=0)
    return du, dlam



# revision 48
# speedup vs baseline: 1.0078x; 1.0015x over previous
"""Trainium2 Bass kernel for nn_AugmentedODE (B=64, N=P=512), 8-core data parallel.

Per batch the reference computes (7 matmuls of 512^3):
    Omega   = 0.5*(A - A^T)
    du      = u @ Omega + G - u @ (u^T G)
    S       = lam @ G^T
    dlam    = lam @ A + (S + S^T) @ u

Restructured to 5 matmuls + 1 PE transpose set per batch:
    UTG = u^T G                      (bf16:  lhsT=u,    rhs=G)
    W   = 0.5*(A - A^T) - UTG        (DVE; A - A^T precomputed host-side, fp8)
    du  = u @ W + G                  (bf16:  lhsT=u^T,  rhs=W; +G fused in DVE)
    S   = lam @ G^T                  (bf16:  lhsT=lam^T, rhs=G^T)
    C   = S + S^T                    (bf16 PE transpose + DVE add)
    dlam= lam @ A + C @ u            (lam@A in fp8 DoubleRow at 2x rate; C@u bf16;
                                      both accumulated into one PSUM group)

Rel-err budget is 2e-2 (Frobenius); measured 1.734e-2 for this mix (du
1.734e-2, dlam 1.689e-2 — deterministic, seed-0 inputs).  The four
magnitude-dominant matmuls (UTG, u@W, S, C@u) stay bf16 for batches 2-7 and
run at the PE's bf16 roofline (1 row/cycle, ~216 ns per 128x128x512 matmul).
Batch 0 is fully fp8 (halves its head DMA) and runs its S matmul + half its
UTG in fp8 DoubleRow (numerically identical to its plain-fp8 form, ~2.4us
PE saved).  Batch 1 runs UTG, S AND u@W in fp8 DoubleRow (~4.8us PE saved);
u8 is shipped pre-scaled by 1/4 (exact) so the on-device W/4 fits fp8e4's
+-240 range (|W| itself reaches ~590 — product-of-Gaussians tails), and the
du add rescales by 4.  lam@A (~3% of |dlam|) and the skew term (~5% of |W|)
are fp8 DR everywhere.  The error budget is now ~87% consumed; converting
any further bf16 matmul instance to fp8 (+~1e-4 variance each) would cross
the 2e-2 gate.

(Note: offloading the S+S^T transposes to the DMA xbar was tried and
reverted — Tile globally serializes dma-transposes against ALL in-flight
HWDGE DMAs as a HW-deadlock guard, which strings them out by 15-25us and
starves the PE.  The 16 PE transposes per batch stream at ~56ns each and
are the cheapest correct option.)

All operands are pre-packed on the host into the exact SBUF layout
([128 partitions, kblock, 512] with k-blocks contiguous per partition) and
concatenated into three blobs per batch, so every DMA line is multi-KB
contiguous on both sides.  Dependency tracking is per-tile, so batch 0 uses
peeled per-piece tiles whose DMAs are sequenced on one queue in consumption
order; batches 1-3 stream whole blobs on the same queue behind them; later
batches prefetch on parallel queues, gated naturally by the 4-deep input
rings.  Batches are processed in PAIRS with sections interleaved
(M1(b), M1(b+1), M5(b), M5(b+1), ...) so every section boundary is followed
by independent work and cross-engine handoff latencies stay off the PE
critical path.  Outputs are written bf16 and upcast on the host.
"""
import numpy as np
import ml_dtypes

import concourse.bass as bass
import concourse.mybir as mybir
import concourse.tile as tile
from concourse import bacc
from concourse.bass_utils import run_bass_kernel_spmd
from concourse.masks import make_identity

F32 = mybir.dt.float32
F32R = mybir.dt.float32r
BF16 = mybir.dt.bfloat16
F8 = mybir.dt.float8e4
AOP = mybir.AluOpType
DR = mybir.MatmulPerfMode.DoubleRow

NP_BF16 = ml_dtypes.bfloat16
NP_F8 = ml_dtypes.float8_e4m3

B, N, P = 64, 512, 512
NCORES = 8
BLOC = B // NCORES          # batches per core
KB = 4                      # 512 = 4 k-blocks of 128
CH = 4                      # 4 output chunks of 128 rows
WARMUP_MM = 6


def _build_nc():
    nc = bacc.Bacc("TRN2", target_bir_lowering=False, debug=False,
                   num_devices=NCORES)

    # in1: interleaved [u0,g0,u1,g1,u2,g2,u3,g3]
    # in2: interleaved [lamt0,gt0,...,lamt3,gt3, ut0..ut3]
    # in8: amat=A-A^T(0:4) | a(4:8) | lamt8(8:12), fp8e4
    d_in1 = nc.declare_dram_parameter("in1", [BLOC, 128, 2 * KB, P], BF16,
                                      isOutput=False)
    d_in2 = nc.declare_dram_parameter("in2", [BLOC, 128, 3 * KB, P], BF16,
                                      isOutput=False)
    d_in8 = nc.declare_dram_parameter("in8", [BLOC, 128, 3 * KB, P], F8,
                                      isOutput=False)
    # batch-0 fp8 head blobs: h2f8 = [gt0..3 | ut0..3] (its lamt fp8 comes
    # from in8's lamt8 blocks, so it is not shipped twice), h1f8 =
    # [u0,g0 | u1,g1 | u2,u3,g2,g3] (last four blocks reordered so the
    # (u2,u3)x(g2,g3) k-pair runs as one fp8 DoubleRow pass)
    d_h2f8 = nc.declare_dram_parameter("h2f8", [128, 2 * KB, P], F8,
                                       isOutput=False)
    d_h1f8 = nc.declare_dram_parameter("h1f8", [128, 2 * KB, P], F8,
                                       isOutput=False)
    # batch-1 fp8 operands for its DoubleRow UTG, S and u@W matmuls:
    # [u8(0:4) | g8(4:8) | gt8(8:12) | ut8(12:16)]
    d_x8b1 = nc.declare_dram_parameter("x8b1", [128, 4 * KB, P], F8,
                                       isOutput=False)
    d_du = nc.declare_dram_parameter("du", [BLOC, 128, KB, P], BF16,
                                     isOutput=True)
    d_dlam = nc.declare_dram_parameter("dlam", [BLOC, 128, KB, P], BF16,
                                       isOutput=True)

    with tile.TileContext(nc) as tc:
        with (
            tc.tile_pool(name="const", bufs=1) as constp,
            tc.tile_pool(name="ins", bufs=4) as insp,
            tc.tile_pool(name="mid", bufs=2) as midp,
            tc.tile_pool(name="outs", bufs=2) as outsp,
            tc.tile_pool(name="psum", bufs=6, space="PSUM") as psum,
        ):
            # warm-up source first: one memset, no identity dependency, so the
            # PE's ~6us low-clock window starts at the earliest possible point
            # memset on DVE: gpsimd is busy with its framework preamble
            # (DMA rings, library loads) until ~7us, which would delay the
            # first warm-up matmul and with it the HAM un-throttle point
            wsrc = constp.tile([128, 512], BF16)
            nc.vector.memset(wsrc[:].bitcast(F32), 0.0)
            warm_ps = psum.tile([128, 512], F32, tag="ps")
            for i in range(WARMUP_MM):
                nc.tensor.matmul(warm_ps[:], wsrc[:, 0:128], wsrc[:],
                                 start=True, stop=True)

            identf = constp.tile([128, 128], F32)
            make_identity(nc, identf[:])
            identb = constp.tile([128, 128], BF16)
            nc.gpsimd.tensor_copy(identb[:], identf[:])

            # batch-0 peeled tiles: per-piece DMAs on ONE queue in consumption
            # order (dep tracking is per-tile, so fine pieces start compute early)
            h1a = insp.tile([128, 2, P], F8, tag="h1a", bufs=1)        # u0,g0
            h1b1 = insp.tile([128, 2, P], F8, tag="h1b1", bufs=1)      # u1,g1
            h1b2 = insp.tile([128, 4, P], F8, tag="h1b2", bufs=1)      # u2,u3,g2,g3
            h8a = insp.tile([128, KB, P], F8, tag="h8a", bufs=1)       # amat
            hgt = insp.tile([128, KB, P], F8, tag="hgt", bufs=1)       # gt0..3
            h8b2 = insp.tile([128, KB, P], F8, tag="h8b2", bufs=1)     # lamt8
            h2c = insp.tile([128, KB, P], F8, tag="h2c", bufs=1)       # ut
            h8b1 = insp.tile([128, KB, P], F8, tag="h8b1", bufs=1)     # a
            # consumption order: M1's u/g pieces, M5-DR's gt8+lamt8, only
            # THEN the W skew term (the DVE STT runs after M1's PSUM drains
            # anyway), M23's ut8, and the tail's a-blocks last.  Putting
            # amat before gt8/lamt8 was measured as a 0.8-2.3us PE stall:
            # M5(b0) sat waiting while bytes it didn't need yet streamed.
            nc.sync.dma_start(h1a[:], d_h1f8[:, 0:2])
            nc.sync.dma_start(h1b1[:], d_h1f8[:, 2:4])
            nc.sync.dma_start(h1b2[:], d_h1f8[:, 4:2 * KB])
            nc.sync.dma_start(hgt[:], d_h2f8[:, 0:KB])
            nc.sync.dma_start(h8b2[:], d_in8[0][:, 2 * KB:3 * KB])
            nc.sync.dma_start(h8a[:], d_in8[0][:, 0:KB])
            nc.sync.dma_start(h2c[:], d_h2f8[:, KB:2 * KB])
            nc.sync.dma_start(h8b1[:], d_in8[0][:, KB:2 * KB])

            def mk_views(b):
                """Allocate input tiles (b>0), emit their DMAs, return accessors."""
                if b == 0:
                    h1 = [h1a, h1b1, h1b2, h1b2]
                    h1o = [0, 0, 0, 1]
                    g1o = [1, 1, 2, 3]
                    return dict(
                        Un=lambda k: h1[k][:, h1o[k]],
                        Gn=lambda k: h1[k][:, g1o[k]],
                        UT=lambda k: h2c[:, k],
                        AM8=lambda k: h8a[:, k],
                        A8pair=lambda j: h8b1[:, 2 * j:2 * j + 2],
                        L8pair=lambda j, r: h8b2[:, 2 * j:2 * j + 2,
                                                 r * 128:(r + 1) * 128],
                        GT8pair=lambda j: hgt[:, 2 * j:2 * j + 2],
                        U8H=lambda r: h1b2[:, 0:2, r * 128:(r + 1) * 128],
                        G8H=lambda: h1b2[:, 2:4],
                    )
                if b == 1:
                    # batch 1 computes UTG, S, AND u@W in fp8 DoubleRow from
                    # x8b1 (= [u8 | g8 | gt8 | ut8]); its bf16 lamt/gt/ut are
                    # never read, nor is its bf16 G (the du "+G" add reads
                    # g8 — G is ~1/512 of |du|, so fp8 G costs nothing).
                    # Only the u half of in1[1] is still loaded in bf16 (the
                    # C@u right operand, which feeds the dominant dlam term).
                    # (Peeling these blobs into finer per-section pieces was
                    # tried and reverted: the head stream is bandwidth-bound,
                    # so pieces only redistribute the same wait while adding
                    # per-piece completion-latency jitter.)
                    in8 = insp.tile([128, 3 * KB, P], F8, tag="in8",
                                    name="in8_1")
                    x8 = insp.tile([128, 4 * KB, P], F8, tag="x8b1", bufs=1)
                    in1u = insp.tile([128, KB, P], BF16, tag="in1b1", bufs=1)
                    nc.sync.dma_start(x8[:], d_x8b1[:])
                    nc.sync.dma_start(in8[:], d_in8[1])
                    nc.sync.dma_start(in1u[:], d_in1[1][:, 0:2 * KB:2])
                    return dict(
                        Un=lambda k, t=in1u: t[:, k],
                        Gn=lambda k, t=x8: t[:, KB + k],
                        AM8=lambda k, t=in8: t[:, k],
                        A8pair=lambda j, t=in8: t[:, KB + 2 * j:KB + 2 * j + 2],
                        L8pair=lambda j, r, t=in8: t[:, 2 * KB + 2 * j:
                                                     2 * KB + 2 * j + 2,
                                                     r * 128:(r + 1) * 128],
                        U8pair=lambda j, r, t=x8: t[:, 2 * j:2 * j + 2,
                                                    r * 128:(r + 1) * 128],
                        G8pair=lambda j, t=x8: t[:, KB + 2 * j:KB + 2 * j + 2],
                        GT8pair=lambda j, t=x8: t[:, 2 * KB + 2 * j:
                                                  2 * KB + 2 * j + 2],
                        UT8pair=lambda j, r, t=x8: t[:, 3 * KB + 2 * j:
                                                     3 * KB + 2 * j + 2,
                                                     r * 128:(r + 1) * 128],
                    )
                # bufs=3: only b2..b7 allocate from these rings (b1 uses
                # private slim tiles), so 3 slots restore the "b5 gated on
                # b2's release" back-pressure that keeps late batches' loads
                # from stealing head DMA bandwidth.
                in1 = insp.tile([128, 2 * KB, P], BF16, tag="in1", bufs=3,
                                name=f"in1_{b}")
                in2 = insp.tile([128, 3 * KB, P], BF16, tag="in2", bufs=3,
                                name=f"in2_{b}")
                in8 = insp.tile([128, 3 * KB, P], F8, tag="in8",
                                name=f"in8_{b}")
                if b <= 4:
                    # head batches: ordered on sync behind batch 0's pieces,
                    # so nothing dilutes the stream batch 0/1 are waiting on
                    nc.sync.dma_start(in1[:], d_in1[b])
                    nc.sync.dma_start(in8[:], d_in8[b])
                    nc.sync.dma_start(in2[:], d_in2[b])
                else:
                    # b>=5 are ring-gated behind live tiles (slot frees only
                    # after batch b-4 completes), so parallel queues can't
                    # steal head bandwidth
                    nc.sync.dma_start(in1[:], d_in1[b])
                    nc.gpsimd.dma_start(in2[:], d_in2[b])
                    nc.gpsimd.dma_start(in8[:], d_in8[b])
                return dict(
                    Un=lambda k, t=in1: t[:, 2 * k],
                    Gn=lambda k, t=in1: t[:, 2 * k + 1],
                    LT=lambda k, t=in2: t[:, 2 * k],
                    GT=lambda k, t=in2: t[:, 2 * k + 1],
                    UT=lambda k, t=in2: t[:, 2 * KB + k],
                    AM8=lambda k, t=in8: t[:, k],
                    A8pair=lambda j, t=in8: t[:, KB + 2 * j:KB + 2 * j + 2],
                    L8pair=lambda j, r, t=in8: t[:, 2 * KB + 2 * j:
                                                 2 * KB + 2 * j + 2,
                                                 r * 128:(r + 1) * 128],
                )

            def sec_m1(b, V):
                """M1: UTG = u^T G (k-outer) ; W = 0.5*amat - UTG (DVE)."""
                if "UT8pair" in V:
                    # batch 1: W is written in fp8 so u@W can run DoubleRow
                    # (W's k-blocks are pair-adjacent in this layout)
                    w_sb = midp.tile([128, KB, P], F8, tag="w8", bufs=1,
                                     name=f"w8_{b}")
                else:
                    w_sb = midp.tile([128, KB, P], BF16, tag="w",
                                     name=f"w_{b}")
                utg = [psum.tile([128, P], F32, tag="ps", name=f"utg{b}_{r}")
                       for r in range(CH)]
                if "U8pair" in V:
                    for j in range(2):
                        for r in range(CH):
                            nc.tensor.matmul(utg[r][:], V["U8pair"](j, r),
                                             V["G8pair"](j), perf_mode=DR,
                                             start=(j == 0), stop=(j == 1))
                elif "U8H" in V:
                    # batch 0 hybrid: k=0,1 as plain fp8 passes (each gated on
                    # a 128KB head piece, so compute starts at first-piece
                    # arrival), then the (u2,u3) k-pair as one DR pass
                    for k in range(2):
                        for r in range(CH):
                            nc.tensor.matmul(utg[r][:],
                                             V["Un"](k)[:, r * 128:(r + 1) * 128],
                                             V["Gn"](k)[:], start=(k == 0),
                                             stop=False)
                    for r in range(CH):
                        nc.tensor.matmul(utg[r][:], V["U8H"](r), V["G8H"](),
                                         perf_mode=DR, start=False, stop=True,
                                         skip_group_check=True)
                else:
                    for k in range(KB):
                        for r in range(CH):
                            nc.tensor.matmul(utg[r][:],
                                             V["Un"](k)[:, r * 128:(r + 1) * 128],
                                             V["Gn"](k)[:], start=(k == 0),
                                             stop=(k == KB - 1))
                # batch 1: u8 is shipped pre-scaled by 1/4 (exact exponent
                # shift), so utg = UTG/4 and W8 = W/4 stays inside fp8e4's
                # +-240 range (|W| itself reaches ~590); the du add scales
                # the matmul result back by 4.
                wscale = 0.125 if "UT8pair" in V else 0.5
                for r in range(CH):
                    nc.vector.scalar_tensor_tensor(w_sb[:, r], V["AM8"](r)[:],
                                                   wscale, utg[r][:], AOP.mult,
                                                   AOP.subtract)
                return w_sb

            def sec_m5(b, V):
                """M5: S = lam @ G^T (k-outer)."""
                s_sb = midp.tile([128, KB, N], BF16, tag="s", name=f"s_{b}")
                s_ps = [psum.tile([128, N], F32, tag="ps", name=f"s{b}_{r}")
                        for r in range(CH)]
                # r-OUTER: s_ps[2]/s_ps[3] can only allocate once the W STTs
                # release utg[0]/utg[1] from the 6-deep PSUM ring; finishing
                # all of r=0,1 first (the two free banks) keeps the PE busy
                # while the DVE STT chain drains (measured 1.8+0.6us stalls
                # with the k-outer order).
                if "GT8pair" in V:
                    for r in range(CH):
                        for j in range(2):
                            nc.tensor.matmul(s_ps[r][:], V["L8pair"](j, r),
                                             V["GT8pair"](j), perf_mode=DR,
                                             start=(j == 0), stop=(j == 1))
                else:
                    for r in range(CH):
                        for k in range(KB):
                            nc.tensor.matmul(s_ps[r][:],
                                             V["LT"](k)[:, r * 128:(r + 1) * 128],
                                             V["GT"](k)[:], start=(k == 0),
                                             stop=(k == KB - 1))
                for r in range(CH):
                    nc.scalar.copy(s_sb[:, r], s_ps[r][:])
                return s_sb

            def sec_m23(b, V, w_sb):
                """M23: du = u @ W + G, stored out."""
                du_sb = outsp.tile([128, KB, P], BF16, tag="du", name=f"du_{b}")
                for r in range(CH):
                    ps = psum.tile([128, P], F32, tag="ps", name=f"du{b}_{r}")
                    if "UT8pair" in V:
                        for j in range(2):
                            nc.tensor.matmul(ps[:], V["UT8pair"](j, r),
                                             w_sb[:, 2 * j:2 * j + 2],
                                             perf_mode=DR, start=(j == 0),
                                             stop=(j == 1))
                    else:
                        for k in range(KB):
                            nc.tensor.matmul(ps[:],
                                             V["UT"](k)[:, r * 128:(r + 1) * 128],
                                             w_sb[:, k], start=(k == 0),
                                             stop=(k == KB - 1))
                    if "UT8pair" in V:
                        nc.vector.scalar_tensor_tensor(du_sb[:, r], ps[:],
                                                       4.0, V["Gn"](r)[:],
                                                       AOP.mult, AOP.add)
                    else:
                        nc.vector.tensor_tensor(du_sb[:, r], ps[:],
                                                V["Gn"](r)[:], AOP.add)
                    if b == BLOC - 1:
                        nc.scalar.dma_start(d_du[b][:, r], du_sb[:, r])
                if b < BLOC - 1:
                    nc.scalar.dma_start(d_du[b], du_sb[:])

            def sec_tail(b, V, s_sb):
                """C = S + S^T, then dlam = lam @ A (fp8 DR) + C @ u."""
                coup_sb = midp.tile([128, KB, N], BF16, tag="coup",
                                    name=f"coup_{b}")
                for r in range(CH):
                    tps = psum.tile([128, N], BF16, tag="tps", bufs=2,
                                    name=f"tps{b}_{r}")
                    for c in range(KB):
                        nc.tensor.transpose(tps[:, c * 128:(c + 1) * 128],
                                            s_sb[:, c, r * 128:(r + 1) * 128],
                                            identb[:])
                    nc.vector.tensor_tensor(coup_sb[:, r], tps[:], s_sb[:, r],
                                            AOP.add)

                dlam_sb = outsp.tile([128, KB, P], BF16, tag="dlam",
                                     name=f"dlam_{b}")
                dlam_ps = [psum.tile([128, P], F32, tag="ps", name=f"dl{b}_{r}")
                           for r in range(CH)]
                for r in range(CH):
                    for j in range(2):
                        nc.tensor.matmul(dlam_ps[r][:], V["L8pair"](j, r),
                                         V["A8pair"](j), perf_mode=DR,
                                         start=(j == 0), stop=False,
                                         skip_group_check=True)
                for r in range(CH):
                    ps = dlam_ps[r]
                    for k in range(KB):
                        nc.tensor.matmul(ps[:],
                                         coup_sb[:, k, r * 128:(r + 1) * 128],
                                         V["Un"](k)[:], start=False,
                                         stop=(k == KB - 1),
                                         skip_group_check=True)
                    # alternate copy engines so the copies drain in parallel;
                    # the LAST chunk's copy goes to the (faster) vector engine
                    # so the final store issues as early as possible
                    if r % 2 == 0:
                        nc.scalar.copy(dlam_sb[:, r], ps[:])
                    else:
                        nc.vector.tensor_copy(dlam_sb[:, r], ps[:])
                    if b == BLOC - 1:
                        if r < CH - 1:
                            qq = nc.sync if r % 2 == 0 else nc.scalar
                            qq.dma_start(d_dlam[b][:, r], dlam_sb[:, r])
                        else:
                            # split the very last store across both HWDGE
                            # rings so its halves drain in parallel and the
                            # end-of-kernel wait sees a 64KB receipt, not 128KB
                            nc.sync.dma_start(d_dlam[b][:, r, 0:256],
                                              dlam_sb[:, r, 0:256])
                            nc.scalar.dma_start(d_dlam[b][:, r, 256:512],
                                                dlam_sb[:, r, 256:512])
                if b < BLOC - 1:
                    nc.scalar.dma_start(d_dlam[b], dlam_sb[:])

            # all batches run SOLO: within one batch the section chain
            # M1 -> M5 -> M23 -> tail already overlaps every cross-engine
            # handoff (W's DVE pass runs under M5, the S copies under M23,
            # the coupling add under the tail's DR matmuls), and solo
            # sequencing needs each batch's inputs ~5us later than pairing —
            # decisive, because the head DMA stream measures fully saturated
            # (~357 GB/s) and the PE otherwise catches up with it around b3.
            for group in [(b,) for b in range(BLOC)]:
                Vs = [mk_views(b) for b in group]
                ws = [sec_m1(b, V) for b, V in zip(group, Vs)]
                ss = [sec_m5(b, V) for b, V in zip(group, Vs)]
                for b, V, w in zip(group, Vs, ws):
                    sec_m23(b, V, w)
                for b, V, s in zip(group, Vs, ss):
                    sec_tail(b, V, s)

    nc.compile()
    return nc


_NC = None


def _pack(x, dt):
    """[BLOC,512,512] -> [BLOC,128,4,512] in SBUF layout (partition-major)."""
    return np.ascontiguousarray(
        x.reshape(BLOC, KB, 128, P).transpose(0, 2, 1, 3).astype(dt))


def _unpack(y):
    """[BLOC,128,4,512] bf16 -> [BLOC,512,512] fp32."""
    return y.transpose(0, 2, 1, 3).reshape(BLOC, N, P).astype(np.float32)


def _make_in_maps(u, lam, A, G):
    u = np.asarray(u, dtype=np.float32)
    lam = np.asarray(lam, dtype=np.float32)
    A = np.asarray(A, dtype=np.float32)
    G = np.asarray(G, dtype=np.float32)

    in_maps = []
    for c in range(NCORES):
        sl = slice(c * BLOC, (c + 1) * BLOC)
        uc, lamc, Ac, Gc = u[sl], lam[sl], A[sl], G[sl]
        At = np.swapaxes(Ac, 1, 2)
        lamt = np.swapaxes(lamc, 1, 2)
        in1 = np.empty((BLOC, 128, 2 * KB, P), dtype=NP_BF16)
        in1[:, :, 0::2] = _pack(uc, NP_BF16)
        in1[:, :, 1::2] = _pack(Gc, NP_BF16)
        in2 = np.empty((BLOC, 128, 3 * KB, P), dtype=NP_BF16)
        in2[:, :, 0:2 * KB:2] = _pack(lamt, NP_BF16)
        in2[:, :, 1:2 * KB:2] = _pack(np.swapaxes(Gc, 1, 2), NP_BF16)
        in2[:, :, 2 * KB:] = _pack(np.swapaxes(uc, 1, 2), NP_BF16)
        # lamt8 is quantized from the bf16 lamt (same value chain as on-device)
        in8 = np.concatenate([_pack(Ac - At, NP_F8), _pack(Ac, NP_F8),
                              _pack(lamt, NP_BF16).astype(NP_F8)], axis=2)
        # batch-1 fp8 DoubleRow operands, quantized from the same bf16 chain:
        # [u8/4 | g8 | gt8 | ut8] — u8 quantized then scaled by 1/4 (an
        # exact exponent shift) so the on-device W/4 fits fp8e4's range
        u8q = in1[1][:, 0::2].astype(NP_F8).astype(np.float32) / 4.0
        x8b1 = np.ascontiguousarray(np.concatenate(
            [u8q, in1[1][:, 1::2].astype(np.float32),
             in2[1][:, 1:2 * KB:2].astype(np.float32),
             in2[1][:, 2 * KB:].astype(np.float32)], axis=1)).astype(NP_F8)
        # h1f8: [u0,g0 | u1,g1 | u2,u3,g2,g3]; h2f8: [gt0..3 | ut0..3]
        h1f8 = np.ascontiguousarray(
            in1[0][:, [0, 1, 2, 3, 4, 6, 5, 7]]).astype(NP_F8)
        h2f8 = np.ascontiguousarray(
            in2[0][:, [1, 3, 5, 7, 8, 9, 10, 11]]).astype(NP_F8)
        in_maps.append({"in1": in1, "in2": in2, "in8": in8, "x8b1": x8b1,
                        "h2f8": h2f8, "h1f8": h1f8})
    return in_maps


def kernel(u, lam, A, G, t=None, **_ignored):
    global _NC
    if _NC is None:
        _NC = _build_nc()
    nc = _NC

    in_maps = _make_in_maps(u, lam, A, G)
    res = run_bass_kernel_spmd(nc, in_maps, list(range(NCORES)))
    du = np.concatenate([_unpack(res.results[c]["du"]) for c in range(NCORES)],
                        axis=0)
    dlam = np.concatenate([_unpack(res.results[c]["dlam"])
                           for c in range(NCORES)], axis=0)
    return du, dlam

